# revision 49
# baseline (speedup 1.0000x reference)
"""Trainium2 Bass kernel for nn_PixelTransformer (v2).

Math notes (derived from the reference semantics, valid for ANY input values):
  * The transformer hidden state is built purely from positional encodings
    (x never enters it), broadcast over the batch.  The attention mixes only
    across the batch axis (head_dim=1), so with identical tokens per batch the
    softmax is uniform and the attention output equals v exactly.  Attention +
    residual therefore folds into a per-layer 5x5 linear map.
  * LayerNorm centering is a linear projection C = I - J/5, foldable into the
    preceding matmuls; LN affine params fold into the following matmuls.
  * LayerNorm is invariant to per-pixel positive scaling, and ReLU commutes
    with it.  The kernel keeps the state in an UNSCALED representation u with
    g_true = (1/std) * u, tracking std per pixel.  The FFN bias becomes a 6th
    stationary row applied against the std row of the state tile.
  * LN1's variance is a quadratic form in the previous state w=[u;std]:
    var+eps = ||L w||^2 with L = sqrtm(M^T M/5 + diag(0..0,eps)) computed on
    the host, so the 6 rows of r = L w are emitted as extra output rows of
    the attention matmul and std1 = sqrt(sum r^2) is ready in parallel with
    the attention output itself.
  * Per layer, two state tiles Ta/Tb [6, NP]: rows 0-4 y (bf16), row 5 std.
    LN2's variance accumulates eps*psv_a (via a copied SBUF row) plus
    sum(y2^2)/5.
  * The 16-step affine flow scan has the closed form
      z = exp(S) * x + sum_j exp(sum_{k>j} sc_k) * t_j
    with sf = exp(sfac) folded into the head weights on the host.
  * Outputs: per-core s-sum [16,1] and z tile [B,NP]; host combines.

Sharding: the N=1024 pixels are split across 8 cores (128 each); all weights
are replicated.  Everything is bf16 on device except PSUM accumulation.
"""

import numpy as np

B, H, W = 32, 32, 32
N = H * W
L, D, FF = 8, 5, 2048
NCORES = 8
NP = N // NCORES          # pixels per core
NCHUNK = FF // 128        # 16 ff chunks of 128
EPS = 1e-5

_PROG = None              # cached compiled Bass program

# ---- bfA column layout (layer stationaries + tokens), rows 0-5, bf16 ----
A_TINIT = 0               # [5, NP]  tok rows (std_init handled via ones1)
A_ATTN = A_TINIT + NP     # [5, 12*L] attn mains: cols 0-5 P6-part, 6-11 r-part
A_ATTN1 = A_ATTN + 12 * L  # [1, 12*L] attn rank-1 rows (over std_prev)
A_PRE = A_ATTN1 + 12 * L  # [6, 5*L]  pre stationaries (folded, over Ta)
A_PSV = A_PRE + 5 * L     # col0: ones6 (psva); col1: 0.2 rows0-4; col2: eps@0
A_ZM = A_PSV + 3          # [1, 6] zmask row0 = [0,0,0,0,0,1]
A_COLS = A_ZM + 6

# ---- bfB column layout (w2 + head + x), bf16 ----
B_W2 = 0                  # [128, 80*L] w2, layer l chunk c at col 80l+5c
B_HS1 = B_W2 + 80 * L     # [5, 16] head W1' stationary (rows 0-4)
B_HB1 = B_HS1 + 16        # [1, 16] head W1' bias row (row 0, over std8)
B_HS2 = B_HB1 + 16        # [16, 32] head W2' stationary (s|t), rows 0-15
B_HB2 = B_HS2 + 32        # [1, 32]  head bias-row stationary (row 0)
B_TRI = B_HB2 + 32        # [16, 17] cols 0-15: tri*sf; col 16: ones*sf
B_XSB = B_TRI + 17        # [32, NP] x shard (rows 0-31)
B_COLS = B_XSB + NP


def _build_program():
    import concourse.bacc as bacc
    import concourse.mybir as mybir
    import concourse.tile as tile
    from bass_rust import add_dep_helper

    f32 = mybir.dt.float32
    bf16 = mybir.dt.bfloat16
    AF = mybir.ActivationFunctionType
    ALU = mybir.AluOpType

    nc = bacc.Bacc(name="pixel_transformer")

    bfA_d = nc.dram_tensor("bfA", [6, A_COLS], bf16, kind="ExternalInput")
    bfB_d = nc.dram_tensor("bfB", [128, B_COLS], bf16, kind="ExternalInput")
    w1_d = [
        nc.dram_tensor(f"w1_{l}", [6, FF], bf16, kind="ExternalInput")
        for l in range(L)
    ]
    outs_d = nc.dram_tensor("outs", [16, 2], f32, kind="ExternalOutput")
    outz_d = nc.dram_tensor("outz", [B, NP], f32, kind="ExternalOutput")

    HP = NP // 2              # pixels per half-stream

    with tile.TileContext(nc) as tc:
        with (
            tc.tile_pool(name="consts", bufs=1) as cp,
            tc.tile_pool(name="work", bufs=2) as wp,
            tc.tile_pool(name="fsb", bufs=2) as fp,
            tc.tile_pool(name="ps", bufs=2, space="PSUM") as pp,
        ):
            bfA = cp.tile([6, A_COLS], bf16)
            nc.sync.dma_start(out=bfA, in_=bfA_d[:, :])
            w1sb = []
            for l in range(L):
                w1sb.append(cp.tile([6, FF], bf16, name=f"w1sb{l}"))
            HF = FF // 2
            nc.sync.dma_start(out=w1sb[0][:, 0:HF], in_=w1_d[0][:, 0:HF])
            nc.sync.dma_start(out=w1sb[0][:, HF:FF], in_=w1_d[0][:, HF:FF])
            for l in range(1, L):
                nc.sync.dma_start(out=w1sb[l], in_=w1_d[l][:, :])
            bfB = cp.tile([128, B_COLS], bf16)
            nc.gpsimd.dma_start(out=bfB, in_=bfB_d[:, :])

            # act-table warm: Sqrt first narrows straight to the sqrt set
            vconstf = cp.tile([1, 1], f32)
            nc.vector.memset(vconstf, 1.0)
            warmt = cp.tile([1, 1], f32)
            warm_insts = [
                nc.scalar.activation(out=warmt, in_=vconstf[0:1, 0:1], func=f)
                for f in (AF.Sqrt,)
            ]
            ones_b16 = cp.tile([1, 16], f32)    # broadcast 1 -> 16 partitions
            nc.vector.memset(ones_b16, 1.0)
            ones16x32 = cp.tile([16, B], bf16)  # col-sum 16 + broadcast -> 32
            nc.vector.memset(ones16x32, 1.0)
            ones_b32 = cp.tile([1, B], f32)     # broadcast 1 -> 32 partitions
            nc.vector.memset(ones_b32, 1.0)
            ones1 = cp.tile([1, NP], bf16)      # std_init == 1 row
            nc.vector.memset(ones1, 1.0)

            # two independent half-streams of HP pixels each
            Ty_prev = [bfA[0:5, A_TINIT + HP * h:A_TINIT + HP * (h + 1)]
                       for h in range(2)]
            std_prev = [ones1[0:1, 0:HP], ones1[0:1, HP:NP]]
            psvb_last = [None, None]
            tags = ["A", "B"]
            for l in range(L):
                sa = A_ATTN + 12 * l
                s1 = A_ATTN1 + 12 * l
                pvx, pyy, Ta_t, Tby_t, stda_t, stdb_t, sqr_t, sqy_t = (
                    [None, None] for _ in range(8))
                psf2 = [None, None]
                # attention r-part, then P6 mains (bankX group 1, bankY gr 1)
                for h in range(2):
                    t = tags[h]
                    pvx[h] = pp.tile([6, 192], f32, tag=f"pvx{t}", bufs=1,
                                     name=f"pvx{l}{t}")
                    pyy[h] = pp.tile([6, 128], f32, tag=f"pyy{t}", bufs=1,
                                     name=f"pyy{l}{t}")
                    at = nc.tensor.matmul(
                        pvx[h][:, 0:HP], bfA[0:5, sa + 6:sa + 12], Ty_prev[h],
                        start=True, stop=False,
                    )
                    nc.tensor.matmul(
                        pyy[h][:, 0:HP], bfA[0:5, sa:sa + 6], Ty_prev[h],
                        start=True, stop=False,
                    )
                    if l == 0:
                        for wi in warm_insts:
                            add_dep_helper(at.ins, wi.ins,
                                           reason="act warm before layer 0")
                # rank-1 parts (need std of the previous layer)
                for h in range(2):
                    nc.tensor.matmul(
                        pvx[h][:, 0:HP], bfA[0:1, s1 + 6:s1 + 12], std_prev[h],
                        start=False, stop=True,
                    )
                    nc.tensor.matmul(
                        pyy[h][:, 0:HP], bfA[0:1, s1:s1 + 6], std_prev[h],
                        start=False, stop=False,
                    )
                # LN1 std chains
                for h in range(2):
                    t = tags[h]
                    sqr_t[h] = wp.tile([6, HP], bf16, tag=f"sqr{t}",
                                       name=f"sqr{l}{t}")
                    nc.scalar.activation(out=sqr_t[h], in_=pvx[h][:, 0:HP],
                                         func=AF.Square)
                for h in range(2):
                    nc.tensor.matmul(pvx[h][0:1, 64:64 + HP],
                                     bfA[0:6, A_PSV:A_PSV + 1], sqr_t[h],
                                     start=True, stop=True)
                for h in range(2):
                    t = tags[h]
                    stda_t[h] = wp.tile([1, HP], bf16, tag=f"sda{t}", bufs=2,
                                        name=f"sda{l}{t}")
                    nc.scalar.activation(out=stda_t[h],
                                         in_=pvx[h][0:1, 64:64 + HP],
                                         func=AF.Sqrt)
                for h in range(2):
                    nc.tensor.matmul(pyy[h][:, 0:HP],
                                     bfA[0:1, A_ZM:A_ZM + 6], stda_t[h],
                                     start=False, stop=True)
                for h in range(2):
                    t = tags[h]
                    Ta_t[h] = wp.tile([6, HP], bf16, tag=f"Ta{t}", bufs=2,
                                      name=f"Ta{l}{t}")
                    nc.vector.tensor_copy(out=Ta_t[h], in_=pyy[h][:, 0:HP])
                # FFN mm1: 16 chunks x HP px -> 2 psum banks per half
                for h in range(2):
                    t = tags[h]
                    psf2[h] = [pp.tile([128, 512], f32, tag=f"pf{t}", bufs=2,
                                       name=f"psf{l}{t}_{q}") for q in range(2)]
                    for c in range(NCHUNK):
                        nc.tensor.matmul(
                            psf2[h][c // 8][:, HP * (c % 8):HP * (c % 8 + 1)],
                            w1sb[l][0:6, 128 * c:128 * (c + 1)],
                            Ta_t[h][0:6, :], start=True, stop=True,
                        )
                    nc.tensor.matmul(
                        pyy[h][0:5, 64:64 + HP],
                        bfA[0:6, A_PRE + 5 * l:A_PRE + 5 * (l + 1)],
                        Ta_t[h][0:6, :], start=True, stop=False,
                    )
                # relu -> bf16 in [128, 256] pieces, balanced ACT/DVE
                fq2 = [None, None]
                for h in range(2):
                    t = tags[h]
                    fq2[h] = [fp.tile([128, 512], bf16, tag=f"f{t}",
                                      name=f"f{l}{t}_{q}") for q in range(2)]
                for h in range(2):
                    for q in range(2):
                        for p in range(2):
                            src_ = psf2[h][q][:, 256 * p:256 * (p + 1)]
                            dst_ = fq2[h][q][:, 256 * p:256 * (p + 1)]
                            if (h + q + p) % 2 == 0:
                                nc.vector.tensor_scalar(
                                    out=dst_, in0=src_, scalar1=0.0,
                                    scalar2=None, op0=ALU.max)
                            else:
                                nc.scalar.activation(out=dst_, in_=src_,
                                                     func=AF.Relu)
                # mm2 accumulation into psy2 region
                for h in range(2):
                    for c in range(NCHUNK):
                        nc.tensor.matmul(
                            pyy[h][0:5, 64:64 + HP],
                            bfB[:, B_W2 + 80 * l + 5 * c:
                                B_W2 + 80 * l + 5 * (c + 1)],
                            fq2[h][c // 8][:, HP * (c % 8):HP * (c % 8 + 1)],
                            start=False, stop=(c == NCHUNK - 1),
                        )
                # LN2 chains
                for h in range(2):
                    t = tags[h]
                    Tby_t[h] = wp.tile([5, HP], bf16, tag=f"Tb{t}", bufs=2,
                                       name=f"Tb{l}{t}")
                    nc.vector.tensor_copy(out=Tby_t[h],
                                          in_=pyy[h][0:5, 64:64 + HP])
                for h in range(2):
                    t = tags[h]
                    sqy_t[h] = wp.tile([5, HP], bf16, tag=f"sqy{t}",
                                       name=f"sqy{l}{t}")
                    nc.gpsimd.tensor_tensor(out=sqy_t[h], in0=Tby_t[h],
                                            in1=Tby_t[h], op=ALU.mult)
                for h in range(2):
                    nc.tensor.matmul(pvx[h][0:1, 128:128 + HP],
                                     bfA[0:6, A_PSV + 2:A_PSV + 3], sqr_t[h],
                                     start=True, stop=False)
                    nc.tensor.matmul(pvx[h][0:1, 128:128 + HP],
                                     bfA[0:5, A_PSV + 1:A_PSV + 2], sqy_t[h],
                                     start=False, stop=True)
                for h in range(2):
                    t = tags[h]
                    stdb_t[h] = wp.tile([1, HP], bf16, tag=f"sdb{t}", bufs=2,
                                        name=f"sdb{l}{t}")
                    nc.scalar.activation(out=stdb_t[h],
                                         in_=pvx[h][0:1, 128:128 + HP],
                                         func=AF.Sqrt)
                    psvb_last[h] = pvx[h][0:1, 128:128 + HP]
                    Ty_prev[h] = Tby_t[h][0:5, :]
                    std_prev[h] = stdb_t[h][0:1, :]

            # ---- head (also split by halves) ----
            std8f = [None, None]
            s8is = []
            for h in range(2):
                t = tags[h]
                std8f[h] = wp.tile([1, HP], f32, tag=f"s8f{t}", name=f"s8f{t}")
                s8is.append(nc.scalar.activation(out=std8f[h],
                                                 in_=psvb_last[h],
                                                 func=AF.Sqrt))
            warm2 = cp.tile([1, 1], f32)
            w2i = nc.scalar.activation(out=warm2, in_=vconstf[0:1, 0:1],
                                       func=AF.Tanh)
            for s8i in s8is:
                add_dep_helper(w2i.ins, s8i.ins,
                               reason="exp table prefetch after sqrts")
            outs_sb = wp.tile([16, 2], f32, tag="ossb")
            zt_full = wp.tile([B, NP], f32, tag="zf")
            for h in range(2):
                t = tags[h]
                rec8 = wp.tile([1, HP], f32, tag=f"rc8{t}")
                nc.vector.reciprocal(out=rec8, in_=std8f[h])
                hps = pp.tile([16, 512], f32, tag=f"pyy{t}", bufs=1,
                               name=f"hps{t}")
                nc.tensor.matmul(hps[:, 0:HP], ones_b16, rec8,
                                 start=True, stop=True)
                rbc = wp.tile([16, HP], f32, tag=f"rbc{t}")
                nc.vector.tensor_copy(out=rbc, in_=hps[:, 0:HP])

                nc.tensor.matmul(hps[:, 64:64 + HP],
                                 bfB[0:5, B_HS1:B_HS1 + 16], Ty_prev[h],
                                 start=True, stop=False)
                nc.tensor.matmul(hps[:, 64:64 + HP],
                                 bfB[0:1, B_HB1:B_HB1 + 16], std_prev[h],
                                 start=False, stop=True)
                hid = wp.tile([16, HP], bf16, tag=f"hid{t}")
                nc.vector.tensor_scalar(out=hid, in0=hps[:, 64:64 + HP],
                                        scalar1=0.0, scalar2=None, op0=ALU.max)
                nc.tensor.matmul(hps[:, 128:128 + HP],
                                 bfB[0:16, B_HS2:B_HS2 + 16], hid,
                                 start=True, stop=False)
                nc.tensor.matmul(hps[:, 128:128 + HP],
                                 bfB[0:1, B_HB2:B_HB2 + 16], std_prev[h],
                                 start=False, stop=True)
                nc.tensor.matmul(hps[:, 192:192 + HP],
                                 bfB[0:16, B_HS2 + 16:B_HS2 + 32], hid,
                                 start=True, stop=False)
                nc.tensor.matmul(hps[:, 192:192 + HP],
                                 bfB[0:1, B_HB2 + 16:B_HB2 + 32], std_prev[h],
                                 start=False, stop=True)

                s_t = wp.tile([16, HP], f32, tag=f"st{t}")
                nc.vector.scalar_tensor_tensor(
                    out=s_t, in0=hps[:, 128:128 + HP], scalar=1.0, in1=rbc,
                    op0=ALU.mult, op1=ALU.mult,
                    accum_out=outs_sb[:, h:h + 1])
                th = wp.tile([16, HP], bf16, tag=f"th{t}")
                nc.scalar.activation(out=th, in_=s_t, func=AF.Tanh)
                t_t = wp.tile([16, HP], f32, tag=f"tt{t}")
                nc.vector.tensor_tensor(out=t_t, in0=hps[:, 192:192 + HP],
                                        in1=rbc, op=ALU.mult)

                hp2 = pp.tile([B, 512], f32, tag=f"pf{t}", bufs=2,
                              name=f"hp2{t}")
                nc.tensor.matmul(hp2[0:16, 0:HP],
                                 bfB[0:16, B_TRI:B_TRI + 16], th,
                                 start=True, stop=True)
                nc.tensor.matmul(hp2[0:1, 64:64 + HP],
                                 bfB[0:16, B_TRI + 16:B_TRI + 17], th,
                                 start=True, stop=True)
                wexp = wp.tile([16, HP], f32, tag=f"wx{t}")
                nc.scalar.activation(out=wexp, in_=hp2[0:16, 0:HP],
                                     func=AF.Exp)
                wt = wp.tile([16, HP], bf16, tag=f"wt{t}")
                nc.vector.tensor_tensor(out=wt, in0=wexp, in1=t_t,
                                        op=ALU.mult)
                nc.tensor.matmul(hp2[0:B, 128:128 + HP], ones16x32, wt,
                                 start=True, stop=True)

                eS = wp.tile([1, HP], f32, tag=f"eS{t}")
                nc.scalar.activation(out=eS, in_=hp2[0:1, 64:64 + HP],
                                     func=AF.Exp)
                nc.tensor.matmul(hp2[0:B, 192:192 + HP], ones_b32, eS,
                                 start=True, stop=True)
                zt = wp.tile([B, HP], f32, tag=f"zt{t}")
                nc.vector.tensor_tensor(
                    out=zt, in0=bfB[0:B, B_XSB + HP * h:B_XSB + HP * (h + 1)],
                    in1=hp2[0:B, 192:192 + HP], op=ALU.mult)
                nc.vector.tensor_tensor(out=zt_full[:, HP * h:HP * (h + 1)],
                                        in0=zt, in1=hp2[0:B, 128:128 + HP],
                                        op=ALU.add)
            nc.sync.dma_start(out=outs_d[:, :], in_=outs_sb)
            nc.sync.dma_start(out=outz_d[:, :], in_=zt_full)

    nc.finalize()
    return nc


def _fold_inputs(inp):
    """Host-side weight folding (float64 for precision, cast at the end)."""
    C = np.eye(D) - np.ones((D, D)) / D
    g = lambda k: np.asarray(inp[k], dtype=np.float64)
    wqkv, bqkv, wo, bo = g("wqkv"), g("bqkv"), g("wo"), g("bo")
    w1, b1, w2, b2 = g("w1"), g("b1"), g("w2"), g("b2")
    ln1w, ln1b, ln2w, ln2b = g("ln1w"), g("ln1b"), g("ln2w"), g("ln2b")

    bfA = np.zeros((6, A_COLS), np.float64)
    bfB = np.zeros((128, B_COLS), np.float64)
    w1l = np.zeros((L, 6, FF), np.float64)

    for l in range(L):
        Dl = np.diag(ln2w[l - 1]) if l > 0 else np.eye(D)
        el = ln2b[l - 1] if l > 0 else np.zeros(D)
        wv = wqkv[l][2 * D:3 * D, :]
        bv = bqkv[l][2 * D:3 * D]
        A0 = np.eye(D) + wo[l] @ wv
        c_attn = wo[l] @ bv + bo[l]
        M = np.zeros((D, 6))
        M[:, 0:5] = C @ A0 @ Dl
        M[:, 5] = C @ (A0 @ el + c_attn)
        # attn stationaries: P6 rows 0-4 = y1 = M w (row 5 zero col);
        # r rows = L w.  Main part over u_prev, rank-1 row over std_prev.
        G = M.T @ M / D
        G[5, 5] += EPS
        ev, Q = np.linalg.eigh(G)
        Lm = (Q * np.sqrt(np.maximum(ev, 0.0))[None, :]) @ Q.T
        sa = A_ATTN + 12 * l
        s1 = A_ATTN1 + 12 * l
        bfA[0:5, sa:sa + 5] = M[:, 0:5].T          # P6 main cols 0-4
        bfA[0:5, sa + 6:sa + 12] = Lm[:, 0:5].T    # r main cols
        bfA[0, s1:s1 + 5] = M[:, 5]                # P6 rank-1 row
        bfA[0, s1 + 6:s1 + 12] = Lm[:, 5]          # r rank-1 row
        # pre stationary: rows 0-4 = (C diag(ln1w)).T ; row 5 = C(ln1b+b2)
        pre = np.zeros((6, 5))
        pre[0:5, :] = (C @ np.diag(ln1w[l])).T
        pre[5, :] = C @ (ln1b[l] + b2[l])
        bfA[0:6, A_PRE + 5 * l:A_PRE + 5 * (l + 1)] = pre
        # w1+b1 rows
        w1l[l, 0:5, :] = (w1[l] * ln1w[l][None, :]).T
        w1l[l, 5, :] = b1[l] + w1[l] @ ln1b[l]
        # w2 chunks: [128, 5] at col 80l+5c
        w2full = (C @ w2[l]).T                      # [FF, 5]
        for c in range(NCHUNK):
            bfB[:, B_W2 + 80 * l + 5 * c:B_W2 + 80 * l + 5 * (c + 1)] = (
                w2full[128 * c:128 * (c + 1), :])

    # psv stationaries + zmask
    bfA[:, A_PSV] = 1.0
    bfA[:, A_PSV + 1] = [.2, .2, .2, .2, .2, 0.0]
    bfA[:, A_PSV + 2] = EPS
    bfA[0, A_ZM + 5] = 1.0

    # head: sf folded on host
    f0w1, f0b1 = g("f0w1"), g("f0b1")
    f0w2, f0b2 = g("f0w2"), g("f0b2")
    sf = float(np.exp(np.float32(np.asarray(inp["sfac"])[0])))
    D8 = np.diag(ln2w[L - 1])
    e8 = ln2b[L - 1]
    bfB[0:5, B_HS1:B_HS1 + 16] = (f0w1 @ D8).T
    bfB[0, B_HB1:B_HB1 + 16] = f0b1 + f0w1 @ e8
    bfB[0:16, B_HS2:B_HS2 + 16] = f0w2.T[:, 0:16] / sf   # s-half, /sf
    bfB[0:16, B_HS2 + 16:B_HS2 + 32] = f0w2.T[:, 16:32]  # t-half
    bfB[0, B_HB2:B_HB2 + 16] = f0b2[0:16] / sf
    bfB[0, B_HB2 + 16:B_HB2 + 32] = f0b2[16:32]
    tri = np.zeros((16, 17))
    for j in range(16):
        tri[j + 1:16, j] = sf                       # sum_{k>j} * sf
    tri[:, 16] = sf
    bfB[0:16, B_TRI:B_TRI + 17] = tri

    # positional tokens, exactly as the reference builds them (fp32 ops)
    xs = (np.arange(W, dtype=np.float32) / np.float32(1e4)).astype(np.float32)
    ys = (np.arange(H, dtype=np.float32) / np.float32(1e4)).astype(np.float32)
    sinx = np.broadcast_to(np.sin(xs)[None, :], (H, W)).reshape(N)
    cosx = np.broadcast_to(np.cos(xs)[None, :], (H, W)).reshape(N)
    siny = np.broadcast_to(np.sin(ys)[:, None], (H, W)).reshape(N)
    cosy = np.broadcast_to(np.cos(ys)[:, None], (H, W)).reshape(N)
    tok = np.stack(
        [-np.ones(N, np.float32), sinx, cosx, siny, cosy], axis=0
    )                                               # [5, N]
    xflat = np.asarray(inp["x"], dtype=np.float32)[:, 0].reshape(B, N)

    return bfA, bfB, w1l, tok, xflat, sf


def get_program():
    global _PROG
    if _PROG is None:
        _PROG = _build_program()
    return _PROG


def make_in_maps(inputs):
    import ml_dtypes

    bfA, bfB, w1l, tok, xflat, sf = _fold_inputs(inputs)
    w1c = {f"w1_{l}": np.ascontiguousarray(w1l[l]).astype(ml_dtypes.bfloat16)
           for l in range(L)}
    in_maps = []
    for core in range(NCORES):
        sl = slice(core * NP, (core + 1) * NP)
        a = bfA.copy()
        a[0:5, A_TINIT:A_TINIT + NP] = tok[:, sl]
        b = bfB.copy()
        b[0:B, B_XSB:B_XSB + NP] = xflat[:, sl]
        m = dict(w1c)
        m["bfA"] = np.ascontiguousarray(a).astype(ml_dtypes.bfloat16)
        m["bfB"] = np.ascontiguousarray(b).astype(ml_dtypes.bfloat16)
        in_maps.append(m)
    return in_maps, sf


def combine_outputs(outs, outzs, sf):
    """per-core s-sums [16,1] (already /sf) and z tiles [B,NP] -> scalar."""
    s_tot = 0.0
    q_tot = 0.0
    for o, oz in zip(outs, outzs):
        s_tot += np.asarray(o, dtype=np.float64).sum()
        q_tot += (np.asarray(oz, dtype=np.float64) ** 2).sum()
    sldj = B * sf * s_tot - 0.5 * q_tot - B * N * 0.5 * np.log(2.0 * np.pi)
    return np.array(-sldj, dtype=np.float32)


def kernel(**inputs):
    from concourse.bass_utils import run_bass_kernel_spmd

    nc = get_program()
    in_maps, sf = make_in_maps(inputs)
    res = run_bass_kernel_spmd(nc, in_maps, core_ids=list(range(NCORES)))
    return combine_outputs([r["outs"] for r in res.results],
                           [r["outz"] for r in res.results], sf)


# revision 54
# speedup vs baseline: 1.0450x; 1.0450x over previous
"""Trainium2 Bass kernel for nn_PixelTransformer (v2).

Math notes (derived from the reference semantics, valid for ANY input values):
  * The transformer hidden state is built purely from positional encodings
    (x never enters it), broadcast over the batch.  The attention mixes only
    across the batch axis (head_dim=1), so with identical tokens per batch the
    softmax is uniform and the attention output equals v exactly.  Attention +
    residual therefore folds into a per-layer 5x5 linear map.
  * LayerNorm centering is a linear projection C = I - J/5, foldable into the
    preceding matmuls; LN affine params fold into the following matmuls.
  * LayerNorm is invariant to per-pixel positive scaling, and ReLU commutes
    with it.  The kernel keeps the state in an UNSCALED representation u with
    g_true = (1/std) * u, tracking std per pixel.  The FFN bias becomes a 6th
    stationary row applied against the std row of the state tile.
  * LN1's variance is a quadratic form in the previous state w=[u;std]:
    var+eps = ||L w||^2 with L = sqrtm(M^T M/5 + diag(0..0,eps)) computed on
    the host, so the 6 rows of r = L w are emitted as extra output rows of
    the attention matmul and std1 = sqrt(sum r^2) is ready in parallel with
    the attention output itself.
  * Per layer, two state tiles Ta/Tb [6, NP]: rows 0-4 y (bf16), row 5 std.
    LN2's variance accumulates eps*psv_a (via a copied SBUF row) plus
    sum(y2^2)/5.
  * The 16-step affine flow scan has the closed form
      z = exp(S) * x + sum_j exp(sum_{k>j} sc_k) * t_j
    with sf = exp(sfac) folded into the head weights on the host.
  * Outputs: per-core s-sum [16,1] and z tile [B,NP]; host combines.

Sharding: the N=1024 pixels are split across 8 cores (128 each); all weights
are replicated.  Everything is bf16 on device except PSUM accumulation.
"""

import numpy as np

B, H, W = 32, 32, 32
N = H * W
L, D, FF = 8, 5, 2048
NCORES = 8
NP = N // NCORES          # pixels per core
NCHUNK = FF // 128        # 16 ff chunks of 128
EPS = 1e-5

_PROG = None              # cached compiled Bass program

# ---- bfA column layout (layer stationaries + tokens), rows 0-5, bf16 ----
A_TINIT = 0               # [5, NP]  tok rows (std_init handled via ones1)
A_ATTN = A_TINIT + NP     # [5, 12*L] attn mains: cols 0-5 P6-part, 6-11 r-part
A_ATTN1 = A_ATTN + 12 * L  # [1, 12*L] attn rank-1 rows (over std_prev)
A_PRE = A_ATTN1 + 12 * L  # [6, 5*L]  pre stationaries (folded, over Ta)
A_PSV = A_PRE + 5 * L     # col0: ones6 (psva); col1: 0.2 rows0-4; col2: eps@0
A_ZM = A_PSV + 3          # [1, 6] zmask row0 = [0,0,0,0,0,1]
A_COLS = A_ZM + 6

# ---- bfB column layout (w2 + head + x), bf16 ----
B_W2 = 0                  # [128, 80*L] w2, layer l chunk c at col 80l+5c
B_HS1 = B_W2 + 80 * L     # [5, 16] head W1' stationary (rows 0-4)
B_HB1 = B_HS1 + 16        # [1, 16] head W1' bias row (row 0, over std8)
B_HS2 = B_HB1 + 16        # [16, 32] head W2' stationary (s|t), rows 0-15
B_HB2 = B_HS2 + 32        # [1, 32]  head bias-row stationary (row 0)
B_TRI = B_HB2 + 32        # [16, 17] cols 0-15: tri*sf; col 16: ones*sf
B_XSB = B_TRI + 17        # [32, NP] x shard (rows 0-31)
B_COLS = B_XSB + NP


def _build_program():
    import concourse.bacc as bacc
    import concourse.mybir as mybir
    import concourse.tile as tile
    from bass_rust import add_dep_helper

    f32 = mybir.dt.float32
    bf16 = mybir.dt.bfloat16
    AF = mybir.ActivationFunctionType
    ALU = mybir.AluOpType

    nc = bacc.Bacc(name="pixel_transformer")

    bfA_d = nc.dram_tensor("bfA", [6, A_COLS], bf16, kind="ExternalInput")
    bfB_d = nc.dram_tensor("bfB", [128, B_COLS], bf16, kind="ExternalInput")
    w1_d = [
        nc.dram_tensor(f"w1_{l}", [6, FF], bf16, kind="ExternalInput")
        for l in range(L)
    ]
    outs_d = nc.dram_tensor("outs", [16, 2], f32, kind="ExternalOutput")
    outz_d = nc.dram_tensor("outz", [1, 2 * NP], f32, kind="ExternalOutput")

    HP = NP // 2              # pixels per half-stream

    with tile.TileContext(nc) as tc:
        with (
            tc.tile_pool(name="consts", bufs=1) as cp,
            tc.tile_pool(name="work", bufs=2) as wp,
            tc.tile_pool(name="fsb", bufs=2) as fp,
            tc.tile_pool(name="ps", bufs=2, space="PSUM") as pp,
        ):
            bfA = cp.tile([6, A_COLS], bf16)
            nc.sync.dma_start(out=bfA, in_=bfA_d[:, :])
            w1sb = []
            for l in range(L):
                w1sb.append(cp.tile([6, FF], bf16, name=f"w1sb{l}"))
            HF = FF // 2
            nc.sync.dma_start(out=w1sb[0][:, 0:HF], in_=w1_d[0][:, 0:HF])
            nc.sync.dma_start(out=w1sb[0][:, HF:FF], in_=w1_d[0][:, HF:FF])
            for l in range(1, L):
                nc.sync.dma_start(out=w1sb[l], in_=w1_d[l][:, :])
            bfB = cp.tile([128, B_COLS], bf16)
            nc.gpsimd.dma_start(out=bfB, in_=bfB_d[:, :])

            # act-table warm: Sqrt first narrows straight to the sqrt set
            vconstf = cp.tile([1, 1], f32)
            nc.vector.memset(vconstf, 1.0)
            warmt = cp.tile([1, 1], f32)
            warm_insts = [
                nc.scalar.activation(out=warmt, in_=vconstf[0:1, 0:1], func=f)
                for f in (AF.Sqrt,)
            ]
            ones_b16 = cp.tile([1, 16], f32)    # broadcast 1 -> 16 partitions
            nc.vector.memset(ones_b16, 1.0)
            ones16x32 = cp.tile([16, B], bf16)  # col-sum 16 + broadcast -> 32
            nc.vector.memset(ones16x32, 1.0)
            ones16 = cp.tile([16, 1], bf16)     # col-sum over 16 partitions
            nc.vector.memset(ones16, 1.0)
            ones1 = cp.tile([1, NP], bf16)      # std_init == 1 row
            nc.vector.memset(ones1, 1.0)

            # two independent half-streams of HP pixels each
            Ty_prev = [bfA[0:5, A_TINIT + HP * h:A_TINIT + HP * (h + 1)]
                       for h in range(2)]
            std_prev = [ones1[0:1, 0:HP], ones1[0:1, HP:NP]]
            psvb_last = [None, None]
            tags = ["A", "B"]
            for l in range(L):
                sa = A_ATTN + 12 * l
                s1 = A_ATTN1 + 12 * l
                pvx, pyy, Ta_t, Tby_t, stda_t, stdb_t, sqr_t, sqy_t = (
                    [None, None] for _ in range(8))
                psf2 = [None, None]
                # attention r-part, then P6 mains (bankX group 1, bankY gr 1)
                for h in range(2):
                    t = tags[h]
                    pvx[h] = pp.tile([6, 192], f32, tag=f"pvx{t}", bufs=1,
                                     name=f"pvx{l}{t}")
                    pyy[h] = pp.tile([6, 128], f32, tag=f"pyy{t}", bufs=1,
                                     name=f"pyy{l}{t}")
                    at = nc.tensor.matmul(
                        pvx[h][:, 0:HP], bfA[0:5, sa + 6:sa + 12], Ty_prev[h],
                        start=True, stop=False,
                    )
                    nc.tensor.matmul(
                        pyy[h][:, 0:HP], bfA[0:5, sa:sa + 6], Ty_prev[h],
                        start=True, stop=False,
                    )
                    if l == 0:
                        for wi in warm_insts:
                            add_dep_helper(at.ins, wi.ins,
                                           reason="act warm before layer 0")
                # rank-1 parts (need std of the previous layer)
                for h in range(2):
                    nc.tensor.matmul(
                        pvx[h][:, 0:HP], bfA[0:1, s1 + 6:s1 + 12], std_prev[h],
                        start=False, stop=True,
                    )
                    nc.tensor.matmul(
                        pyy[h][:, 0:HP], bfA[0:1, s1:s1 + 6], std_prev[h],
                        start=False, stop=False,
                    )
                # LN1 std chains
                for h in range(2):
                    t = tags[h]
                    sqr_t[h] = wp.tile([6, HP], bf16, tag=f"sqr{t}",
                                       name=f"sqr{l}{t}")
                    nc.scalar.activation(out=sqr_t[h], in_=pvx[h][:, 0:HP],
                                         func=AF.Square)
                for h in range(2):
                    nc.tensor.matmul(pvx[h][0:1, 64:64 + HP],
                                     bfA[0:6, A_PSV:A_PSV + 1], sqr_t[h],
                                     start=True, stop=True)
                for h in range(2):
                    t = tags[h]
                    stda_t[h] = wp.tile([1, HP], bf16, tag=f"sda{t}", bufs=2,
                                        name=f"sda{l}{t}")
                    nc.scalar.activation(out=stda_t[h],
                                         in_=pvx[h][0:1, 64:64 + HP],
                                         func=AF.Sqrt)
                for h in range(2):
                    nc.tensor.matmul(pyy[h][:, 0:HP],
                                     bfA[0:1, A_ZM:A_ZM + 6], stda_t[h],
                                     start=False, stop=True)
                for h in range(2):
                    t = tags[h]
                    Ta_t[h] = wp.tile([6, HP], bf16, tag=f"Ta{t}", bufs=2,
                                      name=f"Ta{l}{t}")
                    nc.vector.tensor_copy(out=Ta_t[h], in_=pyy[h][:, 0:HP])
                # FFN mm1: 16 chunks x HP px -> 2 psum banks per half
                for h in range(2):
                    t = tags[h]
                    psf2[h] = [pp.tile([128, 512], f32, tag=f"pf{t}", bufs=2,
                                       name=f"psf{l}{t}_{q}") for q in range(2)]
                    for c in range(NCHUNK):
                        nc.tensor.matmul(
                            psf2[h][c // 8][:, HP * (c % 8):HP * (c % 8 + 1)],
                            w1sb[l][0:6, 128 * c:128 * (c + 1)],
                            Ta_t[h][0:6, :], start=True, stop=True,
                        )
                    nc.tensor.matmul(
                        pyy[h][0:5, 64:64 + HP],
                        bfA[0:6, A_PRE + 5 * l:A_PRE + 5 * (l + 1)],
                        Ta_t[h][0:6, :], start=True, stop=False,
                    )
                # relu -> bf16 in [128, 256] pieces, balanced ACT/DVE
                fq2 = [None, None]
                for h in range(2):
                    t = tags[h]
                    fq2[h] = [fp.tile([128, 512], bf16, tag=f"f{t}",
                                      name=f"f{l}{t}_{q}") for q in range(2)]
                for h in range(2):
                    for q in range(2):
                        for p in range(2):
                            src_ = psf2[h][q][:, 256 * p:256 * (p + 1)]
                            dst_ = fq2[h][q][:, 256 * p:256 * (p + 1)]
                            if (h + q + p) % 2 == 0:
                                nc.vector.tensor_scalar(
                                    out=dst_, in0=src_, scalar1=0.0,
                                    scalar2=None, op0=ALU.max)
                            else:
                                nc.scalar.activation(out=dst_, in_=src_,
                                                     func=AF.Relu)
                # mm2 accumulation into psy2 region
                for h in range(2):
                    for c in range(NCHUNK):
                        nc.tensor.matmul(
                            pyy[h][0:5, 64:64 + HP],
                            bfB[:, B_W2 + 80 * l + 5 * c:
                                B_W2 + 80 * l + 5 * (c + 1)],
                            fq2[h][c // 8][:, HP * (c % 8):HP * (c % 8 + 1)],
                            start=False, stop=(c == NCHUNK - 1),
                        )
                # LN2 chains
                for h in range(2):
                    t = tags[h]
                    sqy_t[h] = wp.tile([5, HP], bf16, tag=f"sqy{t}",
                                       name=f"sqy{l}{t}")
                    nc.scalar.activation(out=sqy_t[h],
                                         in_=pyy[h][0:5, 64:64 + HP],
                                         func=AF.Square)
                    Tby_t[h] = wp.tile([5, HP], bf16, tag=f"Tb{t}", bufs=2,
                                       name=f"Tb{l}{t}")
                    nc.vector.tensor_copy(out=Tby_t[h],
                                          in_=pyy[h][0:5, 64:64 + HP])
                for h in range(2):
                    nc.tensor.matmul(pvx[h][0:1, 128:128 + HP],
                                     bfA[0:6, A_PSV + 2:A_PSV + 3], sqr_t[h],
                                     start=True, stop=False)
                    nc.tensor.matmul(pvx[h][0:1, 128:128 + HP],
                                     bfA[0:5, A_PSV + 1:A_PSV + 2], sqy_t[h],
                                     start=False, stop=True)
                for h in range(2):
                    t = tags[h]
                    stdb_t[h] = wp.tile([1, HP], bf16, tag=f"sdb{t}", bufs=2,
                                        name=f"sdb{l}{t}")
                    nc.scalar.activation(out=stdb_t[h],
                                         in_=pvx[h][0:1, 128:128 + HP],
                                         func=AF.Sqrt)
                    psvb_last[h] = pvx[h][0:1, 128:128 + HP]
                    Ty_prev[h] = Tby_t[h][0:5, :]
                    std_prev[h] = stdb_t[h][0:1, :]

            # ---- head (also split by halves) ----
            std8f = [None, None]
            s8is = []
            for h in range(2):
                t = tags[h]
                std8f[h] = wp.tile([1, HP], f32, tag=f"s8f{t}", name=f"s8f{t}")
                s8is.append(nc.scalar.activation(out=std8f[h],
                                                 in_=psvb_last[h],
                                                 func=AF.Sqrt))
            warm2 = cp.tile([1, 1], f32)
            w2i = nc.scalar.activation(out=warm2, in_=vconstf[0:1, 0:1],
                                       func=AF.Tanh)
            for s8i in s8is:
                add_dep_helper(w2i.ins, s8i.ins,
                               reason="exp table prefetch after sqrts")
            outs_sb = wp.tile([16, 2], f32, tag="ossb")
            zS = wp.tile([1, 2 * NP], f32, tag="zS")
            for h in range(2):
                t = tags[h]
                rec8 = wp.tile([1, HP], f32, tag=f"rc8{t}")
                nc.vector.reciprocal(out=rec8, in_=std8f[h])
                hps = pp.tile([16, 512], f32, tag=f"pyy{t}", bufs=1,
                               name=f"hps{t}")
                nc.tensor.matmul(hps[:, 0:HP], ones_b16, rec8,
                                 start=True, stop=True)
                rbc = wp.tile([16, HP], f32, tag=f"rbc{t}")
                nc.vector.tensor_copy(out=rbc, in_=hps[:, 0:HP])

                nc.tensor.matmul(hps[:, 64:64 + HP],
                                 bfB[0:5, B_HS1:B_HS1 + 16], Ty_prev[h],
                                 start=True, stop=False)
                nc.tensor.matmul(hps[:, 64:64 + HP],
                                 bfB[0:1, B_HB1:B_HB1 + 16], std_prev[h],
                                 start=False, stop=True)
                hid = wp.tile([16, HP], bf16, tag=f"hid{t}")
                nc.vector.tensor_scalar(out=hid, in0=hps[:, 64:64 + HP],
                                        scalar1=0.0, scalar2=None, op0=ALU.max)
                nc.tensor.matmul(hps[:, 128:128 + HP],
                                 bfB[0:16, B_HS2:B_HS2 + 16], hid,
                                 start=True, stop=False)
                nc.tensor.matmul(hps[:, 128:128 + HP],
                                 bfB[0:1, B_HB2:B_HB2 + 16], std_prev[h],
                                 start=False, stop=True)
                nc.tensor.matmul(hps[:, 192:192 + HP],
                                 bfB[0:16, B_HS2 + 16:B_HS2 + 32], hid,
                                 start=True, stop=False)
                nc.tensor.matmul(hps[:, 192:192 + HP],
                                 bfB[0:1, B_HB2 + 16:B_HB2 + 32], std_prev[h],
                                 start=False, stop=True)

                s_t = wp.tile([16, HP], f32, tag=f"st{t}")
                nc.vector.scalar_tensor_tensor(
                    out=s_t, in0=hps[:, 128:128 + HP], scalar=1.0, in1=rbc,
                    op0=ALU.mult, op1=ALU.mult,
                    accum_out=outs_sb[:, h:h + 1])
                th = wp.tile([16, HP], bf16, tag=f"th{t}")
                nc.scalar.activation(out=th, in_=s_t, func=AF.Tanh)
                t_t = wp.tile([16, HP], f32, tag=f"tt{t}")
                nc.vector.tensor_tensor(out=t_t, in0=hps[:, 192:192 + HP],
                                        in1=rbc, op=ALU.mult)

                hp2 = pp.tile([B, 512], f32, tag=f"pf{t}", bufs=2,
                              name=f"hp2{t}")
                nc.tensor.matmul(hp2[0:16, 0:HP],
                                 bfB[0:16, B_TRI:B_TRI + 16], th,
                                 start=True, stop=True)
                nc.tensor.matmul(hp2[0:1, 64:64 + HP],
                                 bfB[0:16, B_TRI + 16:B_TRI + 17], th,
                                 start=True, stop=True)
                wexp = wp.tile([16, HP], f32, tag=f"wx{t}")
                nc.scalar.activation(out=wexp, in_=hp2[0:16, 0:HP],
                                     func=AF.Exp)
                wt = wp.tile([16, HP], bf16, tag=f"wt{t}")
                nc.vector.tensor_tensor(out=wt, in0=wexp, in1=t_t,
                                        op=ALU.mult)
                nc.tensor.matmul(hp2[0:1, 128:128 + HP], ones16, wt,
                                 start=True, stop=True)
                nc.scalar.activation(out=zS[0:1, HP * h:HP * (h + 1)],
                                     in_=hp2[0:1, 64:64 + HP], func=AF.Exp)
                nc.vector.tensor_copy(out=zS[0:1, NP + HP * h:NP + HP * (h + 1)],
                                      in_=hp2[0:1, 128:128 + HP])
            nc.sync.dma_start(out=outs_d[:, :], in_=outs_sb)
            nc.sync.dma_start(out=outz_d[:, :], in_=zS)

    nc.finalize()
    return nc


def _fold_inputs(inp):
    """Host-side weight folding (float64 for precision, cast at the end)."""
    C = np.eye(D) - np.ones((D, D)) / D
    g = lambda k: np.asarray(inp[k], dtype=np.float64)
    wqkv, bqkv, wo, bo = g("wqkv"), g("bqkv"), g("wo"), g("bo")
    w1, b1, w2, b2 = g("w1"), g("b1"), g("w2"), g("b2")
    ln1w, ln1b, ln2w, ln2b = g("ln1w"), g("ln1b"), g("ln2w"), g("ln2b")

    bfA = np.zeros((6, A_COLS), np.float64)
    bfB = np.zeros((128, B_COLS), np.float64)
    w1l = np.zeros((L, 6, FF), np.float64)

    for l in range(L):
        Dl = np.diag(ln2w[l - 1]) if l > 0 else np.eye(D)
        el = ln2b[l - 1] if l > 0 else np.zeros(D)
        wv = wqkv[l][2 * D:3 * D, :]
        bv = bqkv[l][2 * D:3 * D]
        A0 = np.eye(D) + wo[l] @ wv
        c_attn = wo[l] @ bv + bo[l]
        M = np.zeros((D, 6))
        M[:, 0:5] = C @ A0 @ Dl
        M[:, 5] = C @ (A0 @ el + c_attn)
        # attn stationaries: P6 rows 0-4 = y1 = M w (row 5 zero col);
        # r rows = L w.  Main part over u_prev, rank-1 row over std_prev.
        G = M.T @ M / D
        G[5, 5] += EPS
        ev, Q = np.linalg.eigh(G)
        Lm = (Q * np.sqrt(np.maximum(ev, 0.0))[None, :]) @ Q.T
        sa = A_ATTN + 12 * l
        s1 = A_ATTN1 + 12 * l
        bfA[0:5, sa:sa + 5] = M[:, 0:5].T          # P6 main cols 0-4
        bfA[0:5, sa + 6:sa + 12] = Lm[:, 0:5].T    # r main cols
        bfA[0, s1:s1 + 5] = M[:, 5]                # P6 rank-1 row
        bfA[0, s1 + 6:s1 + 12] = Lm[:, 5]          # r rank-1 row
        # pre stationary: rows 0-4 = (C diag(ln1w)).T ; row 5 = C(ln1b+b2)
        pre = np.zeros((6, 5))
        pre[0:5, :] = (C @ np.diag(ln1w[l])).T
        pre[5, :] = C @ (ln1b[l] + b2[l])
        bfA[0:6, A_PRE + 5 * l:A_PRE + 5 * (l + 1)] = pre
        # w1+b1 rows
        w1l[l, 0:5, :] = (w1[l] * ln1w[l][None, :]).T
        w1l[l, 5, :] = b1[l] + w1[l] @ ln1b[l]
        # w2 chunks: [128, 5] at col 80l+5c
        w2full = (C @ w2[l]).T                      # [FF, 5]
        for c in range(NCHUNK):
            bfB[:, B_W2 + 80 * l + 5 * c:B_W2 + 80 * l + 5 * (c + 1)] = (
                w2full[128 * c:128 * (c + 1), :])

    # psv stationaries + zmask
    bfA[:, A_PSV] = 1.0
    bfA[:, A_PSV + 1] = [.2, .2, .2, .2, .2, 0.0]
    bfA[:, A_PSV + 2] = EPS
    bfA[0, A_ZM + 5] = 1.0

    # head: sf folded on host
    f0w1, f0b1 = g("f0w1"), g("f0b1")
    f0w2, f0b2 = g("f0w2"), g("f0b2")
    sf = float(np.exp(np.float32(np.asarray(inp["sfac"])[0])))
    D8 = np.diag(ln2w[L - 1])
    e8 = ln2b[L - 1]
    bfB[0:5, B_HS1:B_HS1 + 16] = (f0w1 @ D8).T
    bfB[0, B_HB1:B_HB1 + 16] = f0b1 + f0w1 @ e8
    bfB[0:16, B_HS2:B_HS2 + 16] = f0w2.T[:, 0:16] / sf   # s-half, /sf
    bfB[0:16, B_HS2 + 16:B_HS2 + 32] = f0w2.T[:, 16:32]  # t-half
    bfB[0, B_HB2:B_HB2 + 16] = f0b2[0:16] / sf
    bfB[0, B_HB2 + 16:B_HB2 + 32] = f0b2[16:32]
    tri = np.zeros((16, 17))
    for j in range(16):
        tri[j + 1:16, j] = sf                       # sum_{k>j} * sf
    tri[:, 16] = sf
    bfB[0:16, B_TRI:B_TRI + 17] = tri

    # positional tokens, exactly as the reference builds them (fp32 ops)
    xs = (np.arange(W, dtype=np.float32) / np.float32(1e4)).astype(np.float32)
    ys = (np.arange(H, dtype=np.float32) / np.float32(1e4)).astype(np.float32)
    sinx = np.broadcast_to(np.sin(xs)[None, :], (H, W)).reshape(N)
    cosx = np.broadcast_to(np.cos(xs)[None, :], (H, W)).reshape(N)
    siny = np.broadcast_to(np.sin(ys)[:, None], (H, W)).reshape(N)
    cosy = np.broadcast_to(np.cos(ys)[:, None], (H, W)).reshape(N)
    tok = np.stack(
        [-np.ones(N, np.float32), sinx, cosx, siny, cosy], axis=0
    )                                               # [5, N]
    xflat = np.asarray(inp["x"], dtype=np.float32)[:, 0].reshape(B, N)

    return bfA, bfB, w1l, tok, xflat, sf


def get_program():
    global _PROG
    if _PROG is None:
        _PROG = _build_program()
    return _PROG


def make_in_maps(inputs):
    import ml_dtypes

    bfA, bfB, w1l, tok, xflat, sf = _fold_inputs(inputs)
    w1c = {f"w1_{l}": np.ascontiguousarray(w1l[l]).astype(ml_dtypes.bfloat16)
           for l in range(L)}
    in_maps = []
    for core in range(NCORES):
        sl = slice(core * NP, (core + 1) * NP)
        a = bfA.copy()
        a[0:5, A_TINIT:A_TINIT + NP] = tok[:, sl]
        b = bfB.copy()
        b[0:B, B_XSB:B_XSB + NP] = xflat[:, sl]
        m = dict(w1c)
        m["bfA"] = np.ascontiguousarray(a).astype(ml_dtypes.bfloat16)
        m["bfB"] = np.ascontiguousarray(b).astype(ml_dtypes.bfloat16)
        in_maps.append(m)
    return in_maps, sf


def combine_outputs(outs, outzs, sf, xflat):
    """per-core s-sums [16,2] (already /sf) and (eS|T) rows -> scalar.
    z = eS * x + T is recomposed on the host per pixel column."""
    s_tot = 0.0
    q_tot = 0.0
    for core, (o, oz) in enumerate(zip(outs, outzs)):
        s_tot += np.asarray(o, dtype=np.float64).sum()
        oz = np.asarray(oz, dtype=np.float64).reshape(2 * N // NCORES)
        eS, T = oz[:N // NCORES], oz[N // NCORES:]
        xs = xflat[:, core * NP:(core + 1) * NP].astype(np.float64)
        z = eS[None, :] * xs + T[None, :]
        q_tot += (z ** 2).sum()
    sldj = B * sf * s_tot - 0.5 * q_tot - B * N * 0.5 * np.log(2.0 * np.pi)
    return np.array(-sldj, dtype=np.float32)


def kernel(**inputs):
    from concourse.bass_utils import run_bass_kernel_spmd

    nc = get_program()
    in_maps, sf = make_in_maps(inputs)
    xflat = np.asarray(inputs["x"], dtype=np.float32)[:, 0].reshape(B, N)
    res = run_bass_kernel_spmd(nc, in_maps, core_ids=list(range(NCORES)))
    return combine_outputs([r["outs"] for r in res.results],
                           [r["outz"] for r in res.results], sf, xflat)


# revision 57
# speedup vs baseline: 1.0489x; 1.0037x over previous
"""Trainium2 Bass kernel for nn_PixelTransformer (v2).

Math notes (derived from the reference semantics, valid for ANY input values):
  * The transformer hidden state is built purely from positional encodings
    (x never enters it), broadcast over the batch.  The attention mixes only
    across the batch axis (head_dim=1), so with identical tokens per batch the
    softmax is uniform and the attention output equals v exactly.  Attention +
    residual therefore folds into a per-layer 5x5 linear map.
  * LayerNorm centering is a linear projection C = I - J/5, foldable into the
    preceding matmuls; LN affine params fold into the following matmuls.
  * LayerNorm is invariant to per-pixel positive scaling, and ReLU commutes
    with it.  The kernel keeps the state in an UNSCALED representation u with
    g_true = (1/std) * u, tracking std per pixel.  The FFN bias becomes a 6th
    stationary row applied against the std row of the state tile.
  * LN1's variance is a quadratic form in the previous state w=[u;std]:
    var+eps = ||L w||^2 with L = sqrtm(M^T M/5 + diag(0..0,eps)) computed on
    the host, so the 6 rows of r = L w are emitted as extra output rows of
    the attention matmul and std1 = sqrt(sum r^2) is ready in parallel with
    the attention output itself.
  * Per layer, two state tiles Ta/Tb [6, NP]: rows 0-4 y (bf16), row 5 std.
    LN2's variance accumulates eps*psv_a (via a copied SBUF row) plus
    sum(y2^2)/5.
  * The 16-step affine flow scan has the closed form
      z = exp(S) * x + sum_j exp(sum_{k>j} sc_k) * t_j
    with sf = exp(sfac) folded into the head weights on the host.
  * Outputs: per-core s-sum [16,1] and z tile [B,NP]; host combines.

Sharding: the N=1024 pixels are split across 8 cores (128 each); all weights
are replicated.  Everything is bf16 on device except PSUM accumulation.
"""

import numpy as np

B, H, W = 32, 32, 32
N = H * W
L, D, FF = 8, 5, 2048
NCORES = 8
NP = N // NCORES          # pixels per core
NCHUNK = FF // 128        # 16 ff chunks of 128
EPS = 1e-5

_PROG = None              # cached compiled Bass program

# ---- bfA column layout (layer stationaries + tokens), rows 0-5, bf16 ----
A_TINIT = 0               # [5, NP]  tok rows (std_init handled via ones1)
A_ATTN = A_TINIT + NP     # [5, 12*L] attn mains: cols 0-5 P6-part, 6-11 r-part
A_ATTN1 = A_ATTN + 12 * L  # [1, 12*L] attn rank-1 rows (over std_prev)
A_PRE = A_ATTN1 + 12 * L  # [6, 5*L]  pre stationaries (folded, over Ta)
A_PSV = A_PRE + 5 * L     # col0: ones6 (psva); col1: 0.2 rows0-4; col2: eps@0
A_ZM = A_PSV + 3          # [1, 6] zmask row0 = [0,0,0,0,0,1]
A_COLS = A_ZM + 6

# ---- bfB column layout (w2 + head + x), bf16 ----
B_W2 = 0                  # [128, 80*L] w2, layer l chunk c at col 80l+5c
B_HS1 = B_W2 + 80 * L     # [5, 16] head W1' stationary (rows 0-4)
B_HB1 = B_HS1 + 16        # [1, 16] head W1' bias row (row 0, over std8)
B_HS2 = B_HB1 + 16        # [16, 32] head W2' stationary (s|t), rows 0-15
B_HB2 = B_HS2 + 32        # [1, 32]  head bias-row stationary (row 0)
B_TRI = B_HB2 + 32        # [16, 17] cols 0-15: tri*sf; col 16: ones*sf
B_XSB = B_TRI + 17        # [32, NP] x shard (rows 0-31)
B_COLS = B_XSB + NP


def _build_program():
    import concourse.bacc as bacc
    import concourse.mybir as mybir
    import concourse.tile as tile
    from bass_rust import add_dep_helper

    f32 = mybir.dt.float32
    bf16 = mybir.dt.bfloat16
    AF = mybir.ActivationFunctionType
    ALU = mybir.AluOpType

    nc = bacc.Bacc(name="pixel_transformer")

    bfA_d = nc.dram_tensor("bfA", [6, A_COLS], bf16, kind="ExternalInput")
    bfB_d = nc.dram_tensor("bfB", [128, B_COLS], bf16, kind="ExternalInput")
    w1_d = [
        nc.dram_tensor(f"w1_{l}", [6, FF], bf16, kind="ExternalInput")
        for l in range(L)
    ]
    outs_d = nc.dram_tensor("outs", [16, 2], f32, kind="ExternalOutput")
    outz_d = nc.dram_tensor("outz", [1, 2 * NP], f32, kind="ExternalOutput")

    HP = NP // 2              # pixels per half-stream

    with tile.TileContext(nc) as tc:
        with (
            tc.tile_pool(name="consts", bufs=1) as cp,
            tc.tile_pool(name="work", bufs=2) as wp,
            tc.tile_pool(name="fsb", bufs=2) as fp,
            tc.tile_pool(name="ps", bufs=2, space="PSUM") as pp,
        ):
            bfA = cp.tile([6, A_COLS], bf16)
            nc.sync.dma_start(out=bfA, in_=bfA_d[:, :])
            w1sb = []
            for l in range(L):
                w1sb.append(cp.tile([6, FF], bf16, name=f"w1sb{l}"))
            HF = FF // 2
            nc.sync.dma_start(out=w1sb[0][:, 0:HF], in_=w1_d[0][:, 0:HF])
            nc.sync.dma_start(out=w1sb[0][:, HF:FF], in_=w1_d[0][:, HF:FF])
            for l in range(1, L):
                nc.sync.dma_start(out=w1sb[l], in_=w1_d[l][:, :])
            bfB = cp.tile([128, B_COLS], bf16)
            nc.gpsimd.dma_start(out=bfB, in_=bfB_d[:, :])

            # act-table warm: Sqrt first narrows straight to the sqrt set
            vconstf = cp.tile([1, 1], f32)
            nc.vector.memset(vconstf, 1.0)
            warmt = cp.tile([1, 1], f32)
            warm_insts = [
                nc.scalar.activation(out=warmt, in_=vconstf[0:1, 0:1], func=f)
                for f in (AF.Sqrt,)
            ]
            ones_b16 = cp.tile([1, 16], f32)    # broadcast 1 -> 16 partitions
            nc.vector.memset(ones_b16, 1.0)
            ones16 = cp.tile([16, 1], bf16)     # col-sum over 16 partitions
            nc.vector.memset(ones16, 1.0)
            ones1 = cp.tile([1, NP], bf16)      # std_init == 1 row
            nc.vector.memset(ones1, 1.0)

            # two independent half-streams of HP pixels each
            Ty_prev = [bfA[0:5, A_TINIT + HP * h:A_TINIT + HP * (h + 1)]
                       for h in range(2)]
            std_prev = [ones1[0:1, 0:HP], ones1[0:1, HP:NP]]
            psvb_last = [None, None]
            tags = ["A", "B"]
            for l in range(L):
                sa = A_ATTN + 12 * l
                s1 = A_ATTN1 + 12 * l
                for h in range(2):
                    t = tags[h]
                    pvx = pp.tile([6, 192], f32, tag=f"pvx{t}", bufs=1,
                                  name=f"pvx{l}{t}")
                    pyy = pp.tile([6, 128], f32, tag=f"pyy{t}", bufs=1,
                                  name=f"pyy{l}{t}")
                    at = nc.tensor.matmul(
                        pvx[:, 0:HP], bfA[0:5, sa + 6:sa + 12], Ty_prev[h],
                        start=True, stop=False,
                    )
                    nc.tensor.matmul(
                        pyy[:, 0:HP], bfA[0:5, sa:sa + 6], Ty_prev[h],
                        start=True, stop=False,
                    )
                    if l == 0:
                        for wi in warm_insts:
                            add_dep_helper(at.ins, wi.ins,
                                           reason="act warm before layer 0")
                    nc.tensor.matmul(
                        pvx[:, 0:HP], bfA[0:1, s1 + 6:s1 + 12], std_prev[h],
                        start=False, stop=True,
                    )
                    nc.tensor.matmul(
                        pyy[:, 0:HP], bfA[0:1, s1:s1 + 6], std_prev[h],
                        start=False, stop=False,
                    )
                    # LN1 std chain
                    sqr = wp.tile([6, HP], bf16, tag=f"sqr{t}",
                                  name=f"sqr{l}{t}")
                    nc.scalar.activation(out=sqr, in_=pvx[:, 0:HP],
                                         func=AF.Square)
                    nc.tensor.matmul(pvx[0:1, 64:64 + HP],
                                     bfA[0:6, A_PSV:A_PSV + 1], sqr,
                                     start=True, stop=True)
                    stda = wp.tile([1, HP], bf16, tag=f"sda{t}", bufs=2,
                                   name=f"sda{l}{t}")
                    nc.scalar.activation(out=stda, in_=pvx[0:1, 64:64 + HP],
                                         func=AF.Sqrt)
                    nc.tensor.matmul(pyy[:, 0:HP],
                                     bfA[0:1, A_ZM:A_ZM + 6], stda,
                                     start=False, stop=True)
                    Ta = wp.tile([6, HP], bf16, tag=f"Ta{t}", bufs=2,
                                 name=f"Ta{l}{t}")
                    nc.vector.tensor_copy(out=Ta, in_=pyy[:, 0:HP])
                    # FFN mm1: 16 chunks -> 2 psum banks
                    psf2 = [pp.tile([128, 512], f32, tag=f"pf{t}", bufs=2,
                                    name=f"psf{l}{t}_{q}") for q in range(2)]
                    for c in range(NCHUNK):
                        nc.tensor.matmul(
                            psf2[c // 8][:, HP * (c % 8):HP * (c % 8 + 1)],
                            w1sb[l][0:6, 128 * c:128 * (c + 1)],
                            Ta[0:6, :], start=True, stop=True,
                        )
                    nc.tensor.matmul(
                        pyy[0:5, 64:64 + HP],
                        bfA[0:6, A_PRE + 5 * l:A_PRE + 5 * (l + 1)],
                        Ta[0:6, :], start=True, stop=False,
                    )
                    # relu -> bf16 in [128, 256] pieces, alternating ACT/DVE
                    fq2 = [fp.tile([128, 512], bf16, tag=f"f{t}",
                                   name=f"f{l}{t}_{q}") for q in range(2)]
                    for q in range(2):
                        for p in range(2):
                            src_ = psf2[q][:, 256 * p:256 * (p + 1)]
                            dst_ = fq2[q][:, 256 * p:256 * (p + 1)]
                            if (h + q + p) % 2 == 0:
                                nc.vector.tensor_scalar(
                                    out=dst_, in0=src_, scalar1=0.0,
                                    scalar2=None, op0=ALU.max)
                            else:
                                nc.scalar.activation(out=dst_, in_=src_,
                                                     func=AF.Relu)
                    # mm2 accumulation into psy2 region
                    for c in range(NCHUNK):
                        nc.tensor.matmul(
                            pyy[0:5, 64:64 + HP],
                            bfB[:, B_W2 + 80 * l + 5 * c:
                                B_W2 + 80 * l + 5 * (c + 1)],
                            fq2[c // 8][:, HP * (c % 8):HP * (c % 8 + 1)],
                            start=False, stop=(c == NCHUNK - 1),
                        )
                    # LN2 chain
                    sqy = wp.tile([5, HP], bf16, tag=f"sqy{t}",
                                  name=f"sqy{l}{t}")
                    nc.scalar.activation(out=sqy, in_=pyy[0:5, 64:64 + HP],
                                         func=AF.Square)
                    Tby = wp.tile([5, HP], bf16, tag=f"Tb{t}", bufs=2,
                                  name=f"Tb{l}{t}")
                    nc.vector.tensor_copy(out=Tby, in_=pyy[0:5, 64:64 + HP])
                    nc.tensor.matmul(pvx[0:1, 128:128 + HP],
                                     bfA[0:6, A_PSV + 2:A_PSV + 3], sqr,
                                     start=True, stop=False)
                    nc.tensor.matmul(pvx[0:1, 128:128 + HP],
                                     bfA[0:5, A_PSV + 1:A_PSV + 2], sqy,
                                     start=False, stop=True)
                    stdb = wp.tile([1, HP], bf16, tag=f"sdb{t}", bufs=2,
                                   name=f"sdb{l}{t}")
                    nc.scalar.activation(out=stdb,
                                         in_=pvx[0:1, 128:128 + HP],
                                         func=AF.Sqrt)
                    psvb_last[h] = pvx[0:1, 128:128 + HP]
                    Ty_prev[h] = Tby[0:5, :]
                    std_prev[h] = stdb[0:1, :]

            # ---- head (also split by halves) ----
            std8f = [None, None]
            s8is = []
            for h in range(2):
                t = tags[h]
                std8f[h] = wp.tile([1, HP], f32, tag=f"s8f{t}", name=f"s8f{t}")
                s8is.append(nc.scalar.activation(out=std8f[h],
                                                 in_=psvb_last[h],
                                                 func=AF.Sqrt))
            warm2 = cp.tile([1, 1], f32)
            w2i = nc.scalar.activation(out=warm2, in_=vconstf[0:1, 0:1],
                                       func=AF.Tanh)
            for s8i in s8is:
                add_dep_helper(w2i.ins, s8i.ins,
                               reason="exp table prefetch after sqrts")
            outs_sb = wp.tile([16, 2], f32, tag="ossb")
            zS = wp.tile([1, 2 * NP], f32, tag="zS")
            for h in range(2):
                t = tags[h]
                rec8 = wp.tile([1, HP], f32, tag=f"rc8{t}")
                nc.vector.reciprocal(out=rec8, in_=std8f[h])
                hps = pp.tile([16, 512], f32, tag=f"pyy{t}", bufs=1,
                               name=f"hps{t}")
                nc.tensor.matmul(hps[:, 0:HP], ones_b16, rec8,
                                 start=True, stop=True)
                rbc = wp.tile([16, HP], f32, tag=f"rbc{t}")
                nc.vector.tensor_copy(out=rbc, in_=hps[:, 0:HP])

                nc.tensor.matmul(hps[:, 64:64 + HP],
                                 bfB[0:5, B_HS1:B_HS1 + 16], Ty_prev[h],
                                 start=True, stop=False)
                nc.tensor.matmul(hps[:, 64:64 + HP],
                                 bfB[0:1, B_HB1:B_HB1 + 16], std_prev[h],
                                 start=False, stop=True)
                hid = wp.tile([16, HP], bf16, tag=f"hid{t}")
                nc.vector.tensor_scalar(out=hid, in0=hps[:, 64:64 + HP],
                                        scalar1=0.0, scalar2=None, op0=ALU.max)
                nc.tensor.matmul(hps[:, 128:128 + HP],
                                 bfB[0:16, B_HS2:B_HS2 + 16], hid,
                                 start=True, stop=False)
                nc.tensor.matmul(hps[:, 128:128 + HP],
                                 bfB[0:1, B_HB2:B_HB2 + 16], std_prev[h],
                                 start=False, stop=True)
                nc.tensor.matmul(hps[:, 192:192 + HP],
                                 bfB[0:16, B_HS2 + 16:B_HS2 + 32], hid,
                                 start=True, stop=False)
                nc.tensor.matmul(hps[:, 192:192 + HP],
                                 bfB[0:1, B_HB2 + 16:B_HB2 + 32], std_prev[h],
                                 start=False, stop=True)

                s_t = wp.tile([16, HP], f32, tag=f"st{t}")
                nc.vector.scalar_tensor_tensor(
                    out=s_t, in0=hps[:, 128:128 + HP], scalar=1.0, in1=rbc,
                    op0=ALU.mult, op1=ALU.mult,
                    accum_out=outs_sb[:, h:h + 1])
                th = wp.tile([16, HP], bf16, tag=f"th{t}")
                nc.scalar.activation(out=th, in_=s_t, func=AF.Tanh)
                t_t = wp.tile([16, HP], f32, tag=f"tt{t}")
                nc.vector.tensor_tensor(out=t_t, in0=hps[:, 192:192 + HP],
                                        in1=rbc, op=ALU.mult)

                hp2 = pp.tile([B, 512], f32, tag=f"pf{t}", bufs=2,
                              name=f"hp2{t}")
                nc.tensor.matmul(hp2[0:16, 0:HP],
                                 bfB[0:16, B_TRI:B_TRI + 16], th,
                                 start=True, stop=True)
                nc.tensor.matmul(hp2[0:1, 64:64 + HP],
                                 bfB[0:16, B_TRI + 16:B_TRI + 17], th,
                                 start=True, stop=True)
                wexp = wp.tile([16, HP], f32, tag=f"wx{t}")
                nc.scalar.activation(out=wexp, in_=hp2[0:16, 0:HP],
                                     func=AF.Exp)
                wt = wp.tile([16, HP], bf16, tag=f"wt{t}")
                nc.vector.tensor_tensor(out=wt, in0=wexp, in1=t_t,
                                        op=ALU.mult)
                nc.tensor.matmul(hp2[0:1, 128:128 + HP], ones16, wt,
                                 start=True, stop=True)
                nc.scalar.activation(out=zS[0:1, HP * h:HP * (h + 1)],
                                     in_=hp2[0:1, 64:64 + HP], func=AF.Exp)
                nc.vector.tensor_copy(out=zS[0:1, NP + HP * h:NP + HP * (h + 1)],
                                      in_=hp2[0:1, 128:128 + HP])
            nc.sync.dma_start(out=outs_d[:, :], in_=outs_sb)
            nc.sync.dma_start(out=outz_d[:, :], in_=zS)

    nc.finalize()
    return nc


def _fold_inputs(inp):
    """Host-side weight folding (float64 for precision, cast at the end)."""
    C = np.eye(D) - np.ones((D, D)) / D
    g = lambda k: np.asarray(inp[k], dtype=np.float64)
    wqkv, bqkv, wo, bo = g("wqkv"), g("bqkv"), g("wo"), g("bo")
    w1, b1, w2, b2 = g("w1"), g("b1"), g("w2"), g("b2")
    ln1w, ln1b, ln2w, ln2b = g("ln1w"), g("ln1b"), g("ln2w"), g("ln2b")

    bfA = np.zeros((6, A_COLS), np.float64)
    bfB = np.zeros((128, B_COLS), np.float64)
    w1l = np.zeros((L, 6, FF), np.float64)

    for l in range(L):
        Dl = np.diag(ln2w[l - 1]) if l > 0 else np.eye(D)
        el = ln2b[l - 1] if l > 0 else np.zeros(D)
        wv = wqkv[l][2 * D:3 * D, :]
        bv = bqkv[l][2 * D:3 * D]
        A0 = np.eye(D) + wo[l] @ wv
        c_attn = wo[l] @ bv + bo[l]
        M = np.zeros((D, 6))
        M[:, 0:5] = C @ A0 @ Dl
        M[:, 5] = C @ (A0 @ el + c_attn)
        # attn stationaries: P6 rows 0-4 = y1 = M w (row 5 zero col);
        # r rows = L w.  Main part over u_prev, rank-1 row over std_prev.
        G = M.T @ M / D
        G[5, 5] += EPS
        ev, Q = np.linalg.eigh(G)
        Lm = (Q * np.sqrt(np.maximum(ev, 0.0))[None, :]) @ Q.T
        sa = A_ATTN + 12 * l
        s1 = A_ATTN1 + 12 * l
        bfA[0:5, sa:sa + 5] = M[:, 0:5].T          # P6 main cols 0-4
        bfA[0:5, sa + 6:sa + 12] = Lm[:, 0:5].T    # r main cols
        bfA[0, s1:s1 + 5] = M[:, 5]                # P6 rank-1 row
        bfA[0, s1 + 6:s1 + 12] = Lm[:, 5]          # r rank-1 row
        # pre stationary: rows 0-4 = (C diag(ln1w)).T ; row 5 = C(ln1b+b2)
        pre = np.zeros((6, 5))
        pre[0:5, :] = (C @ np.diag(ln1w[l])).T
        pre[5, :] = C @ (ln1b[l] + b2[l])
        bfA[0:6, A_PRE + 5 * l:A_PRE + 5 * (l + 1)] = pre
        # w1+b1 rows
        w1l[l, 0:5, :] = (w1[l] * ln1w[l][None, :]).T
        w1l[l, 5, :] = b1[l] + w1[l] @ ln1b[l]
        # w2 chunks: [128, 5] at col 80l+5c
        w2full = (C @ w2[l]).T                      # [FF, 5]
        for c in range(NCHUNK):
            bfB[:, B_W2 + 80 * l + 5 * c:B_W2 + 80 * l + 5 * (c + 1)] = (
                w2full[128 * c:128 * (c + 1), :])

    # psv stationaries + zmask
    bfA[:, A_PSV] = 1.0
    bfA[:, A_PSV + 1] = [.2, .2, .2, .2, .2, 0.0]
    bfA[:, A_PSV + 2] = EPS
    bfA[0, A_ZM + 5] = 1.0

    # head: sf folded on host
    f0w1, f0b1 = g("f0w1"), g("f0b1")
    f0w2, f0b2 = g("f0w2"), g("f0b2")
    sf = float(np.exp(np.float32(np.asarray(inp["sfac"])[0])))
    D8 = np.diag(ln2w[L - 1])
    e8 = ln2b[L - 1]
    bfB[0:5, B_HS1:B_HS1 + 16] = (f0w1 @ D8).T
    bfB[0, B_HB1:B_HB1 + 16] = f0b1 + f0w1 @ e8
    bfB[0:16, B_HS2:B_HS2 + 16] = f0w2.T[:, 0:16] / sf   # s-half, /sf
    bfB[0:16, B_HS2 + 16:B_HS2 + 32] = f0w2.T[:, 16:32]  # t-half
    bfB[0, B_HB2:B_HB2 + 16] = f0b2[0:16] / sf
    bfB[0, B_HB2 + 16:B_HB2 + 32] = f0b2[16:32]
    tri = np.zeros((16, 17))
    for j in range(16):
        tri[j + 1:16, j] = sf                       # sum_{k>j} * sf
    tri[:, 16] = sf
    bfB[0:16, B_TRI:B_TRI + 17] = tri

    # positional tokens, exactly as the reference builds them (fp32 ops)
    xs = (np.arange(W, dtype=np.float32) / np.float32(1e4)).astype(np.float32)
    ys = (np.arange(H, dtype=np.float32) / np.float32(1e4)).astype(np.float32)
    sinx = np.broadcast_to(np.sin(xs)[None, :], (H, W)).reshape(N)
    cosx = np.broadcast_to(np.cos(xs)[None, :], (H, W)).reshape(N)
    siny = np.broadcast_to(np.sin(ys)[:, None], (H, W)).reshape(N)
    cosy = np.broadcast_to(np.cos(ys)[:, None], (H, W)).reshape(N)
    tok = np.stack(
        [-np.ones(N, np.float32), sinx, cosx, siny, cosy], axis=0
    )                                               # [5, N]
    xflat = np.asarray(inp["x"], dtype=np.float32)[:, 0].reshape(B, N)

    return bfA, bfB, w1l, tok, xflat, sf


def get_program():
    global _PROG
    if _PROG is None:
        _PROG = _build_program()
    return _PROG


def make_in_maps(inputs):
    import ml_dtypes

    bfA, bfB, w1l, tok, xflat, sf = _fold_inputs(inputs)
    w1c = {f"w1_{l}": np.ascontiguousarray(w1l[l]).astype(ml_dtypes.bfloat16)
           for l in range(L)}
    in_maps = []
    for core in range(NCORES):
        sl = slice(core * NP, (core + 1) * NP)
        a = bfA.copy()
        a[0:5, A_TINIT:A_TINIT + NP] = tok[:, sl]
        b = bfB.copy()
        b[0:B, B_XSB:B_XSB + NP] = xflat[:, sl]
        m = dict(w1c)
        m["bfA"] = np.ascontiguousarray(a).astype(ml_dtypes.bfloat16)
        m["bfB"] = np.ascontiguousarray(b).astype(ml_dtypes.bfloat16)
        in_maps.append(m)
    return in_maps, sf


def combine_outputs(outs, outzs, sf, xflat):
    """per-core s-sums [16,2] (already /sf) and (eS|T) rows -> scalar.
    z = eS * x + T is recomposed on the host per pixel column."""
    s_tot = 0.0
    q_tot = 0.0
    for core, (o, oz) in enumerate(zip(outs, outzs)):
        s_tot += np.asarray(o, dtype=np.float64).sum()
        oz = np.asarray(oz, dtype=np.float64).reshape(2 * N // NCORES)
        eS, T = oz[:N // NCORES], oz[N // NCORES:]
        xs = xflat[:, core * NP:(core + 1) * NP].astype(np.float64)
        z = eS[None, :] * xs + T[None, :]
        q_tot += (z ** 2).sum()
    sldj = B * sf * s_tot - 0.5 * q_tot - B * N * 0.5 * np.log(2.0 * np.pi)
    return np.array(-sldj, dtype=np.float32)


def kernel(**inputs):
    from concourse.bass_utils import run_bass_kernel_spmd

    nc = get_program()
    in_maps, sf = make_in_maps(inputs)
    xflat = np.asarray(inputs["x"], dtype=np.float32)[:, 0].reshape(B, N)
    res = run_bass_kernel_spmd(nc, in_maps, core_ids=list(range(NCORES)))
    return combine_outputs([r["outs"] for r in res.results],
                           [r["outz"] for r in res.results], sf, xflat)


# revision 63
# speedup vs baseline: 1.0613x; 1.0118x over previous
"""Trainium2 Bass kernel for nn_PixelTransformer (v2).

Math notes (derived from the reference semantics, valid for ANY input values):
  * The transformer hidden state is built purely from positional encodings
    (x never enters it), broadcast over the batch.  The attention mixes only
    across the batch axis (head_dim=1), so with identical tokens per batch the
    softmax is uniform and the attention output equals v exactly.  Attention +
    residual therefore folds into a per-layer 5x5 linear map.
  * LayerNorm centering is a linear projection C = I - J/5, foldable into the
    preceding matmuls; LN affine params fold into the following matmuls.
  * LayerNorm is invariant to per-pixel positive scaling, and ReLU commutes
    with it.  The kernel keeps the state in an UNSCALED representation u with
    g_true = (1/std) * u, tracking std per pixel.  The FFN bias becomes a 6th
    stationary row applied against the std row of the state tile.
  * LN1's variance is a quadratic form in the previous state w=[u;std]:
    var+eps = ||L w||^2 with L = sqrtm(M^T M/5 + diag(0..0,eps)) computed on
    the host, so the 6 rows of r = L w are emitted as extra output rows of
    the attention matmul and std1 = sqrt(sum r^2) is ready in parallel with
    the attention output itself.
  * Per layer, two state tiles Ta/Tb [6, NP]: rows 0-4 y (bf16), row 5 std.
    LN2's variance accumulates eps*psv_a (via a copied SBUF row) plus
    sum(y2^2)/5.
  * The 16-step affine flow scan has the closed form
      z = exp(S) * x + sum_j exp(sum_{k>j} sc_k) * t_j
    with sf = exp(sfac) folded into the head weights on the host.
  * Outputs: per-core s-sum [16,1] and z tile [B,NP]; host combines.

Sharding: the N=1024 pixels are split across 8 cores (128 each); all weights
are replicated.  Everything is bf16 on device except PSUM accumulation.
"""

import numpy as np

B, H, W = 32, 32, 32
N = H * W
L, D, FF = 8, 5, 2048
NCORES = 8
NP = N // NCORES          # pixels per core
NCHUNK = FF // 128        # 16 ff chunks of 128
EPS = 1e-5

_PROG = None              # cached compiled Bass program

# ---- bfA column layout (layer stationaries + tokens), rows 0-5, bf16 ----
A_TINIT = 0               # [5, NP]  tok rows (std_init handled via ones1)
A_ATTN = A_TINIT + NP     # [5, 12*L] attn mains: cols 0-5 P6-part, 6-11 r-part
A_ATTN1 = A_ATTN + 12 * L  # [1, 12*L] attn rank-1 rows (over std_prev)
A_PRE = A_ATTN1 + 12 * L  # [6, 5*L]  pre stationaries (folded, over Ta)
A_PSV = A_PRE + 5 * L     # col0: ones6 (psva); col1: 0.2 rows0-4; col2: eps@0
A_ZM = A_PSV + 3          # [1, 6] zmask row0 = [0,0,0,0,0,1]
A_COLS = A_ZM + 6

# ---- bfB column layout (w2 + head + x), bf16 ----
B_W2 = 0                  # [128, 80*L] w2, layer l chunk c at col 80l+5c
B_HS1 = B_W2 + 80 * L     # [5, 16] head W1' stationary (rows 0-4)
B_HB1 = B_HS1 + 16        # [1, 16] head W1' bias row (row 0, over std8)
B_HS2 = B_HB1 + 16        # [16, 32] head W2' stationary (s|t), rows 0-15
B_HB2 = B_HS2 + 32        # [1, 32]  head bias-row stationary (row 0)
B_TRI = B_HB2 + 32        # [16, 17] cols 0-15: tri*sf; col 16: ones*sf
B_XSB = B_TRI + 17        # [32, NP] x shard (rows 0-31)
B_COLS = B_XSB + NP


def _build_program():
    import concourse.bacc as bacc
    import concourse.mybir as mybir
    import concourse.tile as tile
    from bass_rust import add_dep_helper

    f32 = mybir.dt.float32
    bf16 = mybir.dt.bfloat16
    AF = mybir.ActivationFunctionType
    ALU = mybir.AluOpType

    nc = bacc.Bacc(name="pixel_transformer")

    bfA_d = nc.dram_tensor("bfA", [6, A_COLS], bf16, kind="ExternalInput")
    bfB_d = nc.dram_tensor("bfB", [128, B_COLS], bf16, kind="ExternalInput")
    w1_d = [
        nc.dram_tensor(f"w1_{l}", [6, FF], bf16, kind="ExternalInput")
        for l in range(L)
    ]
    outs_d = nc.dram_tensor("outs", [16, 2], f32, kind="ExternalOutput")
    outz_d = nc.dram_tensor("outz", [1, 2 * NP], f32, kind="ExternalOutput")

    HP = NP // 2              # pixels per half-stream

    with tile.TileContext(nc) as tc:
        with (
            tc.tile_pool(name="consts", bufs=1) as cp,
            tc.tile_pool(name="work", bufs=2) as wp,
            tc.tile_pool(name="fsb", bufs=2) as fp,
            tc.tile_pool(name="ps", bufs=2, space="PSUM") as pp,
        ):
            bfA = cp.tile([6, A_COLS], bf16)
            nc.sync.dma_start(out=bfA, in_=bfA_d[:, :])
            w1sb = []
            for l in range(L):
                w1sb.append(cp.tile([6, FF], bf16, name=f"w1sb{l}"))
            HF = FF // 2
            nc.sync.dma_start(out=w1sb[0][:, 0:HF], in_=w1_d[0][:, 0:HF])
            nc.sync.dma_start(out=w1sb[0][:, HF:FF], in_=w1_d[0][:, HF:FF])
            for l in range(1, L):
                nc.sync.dma_start(out=w1sb[l], in_=w1_d[l][:, :])
            bfB = cp.tile([128, B_COLS], bf16)
            nc.gpsimd.dma_start(out=bfB, in_=bfB_d[:, :])

            # act-table warm: Sqrt first narrows straight to the sqrt set
            vconstf = cp.tile([1, 1], f32)
            nc.vector.memset(vconstf, 1.0)
            warmt = cp.tile([1, 1], f32)
            warm_insts = [
                nc.scalar.activation(out=warmt, in_=vconstf[0:1, 0:1], func=f)
                for f in (AF.Sqrt,)
            ]
            ones_b16 = cp.tile([1, 16], f32)    # broadcast 1 -> 16 partitions
            nc.vector.memset(ones_b16, 1.0)
            ones16 = cp.tile([16, 1], bf16)     # col-sum over 16 partitions
            nc.vector.memset(ones16, 1.0)
            ones1 = cp.tile([1, NP], bf16)      # std_init == 1 row
            nc.vector.memset(ones1, 1.0)

            # two independent half-streams of HP pixels each
            Ty_prev = [bfA[0:5, A_TINIT + HP * h:A_TINIT + HP * (h + 1)]
                       for h in range(2)]
            std_prev = [ones1[0:1, 0:HP], ones1[0:1, HP:NP]]
            psvb_last = [None, None]
            last_sqrts = []
            tags = ["A", "B"]
            for l in range(L):
                sa = A_ATTN + 12 * l
                s1 = A_ATTN1 + 12 * l
                for h in range(2):
                    t = tags[h]
                    pvx = pp.tile([6, 192], f32, tag=f"pvx{t}", bufs=1,
                                  name=f"pvx{l}{t}")
                    pyy = pp.tile([6, 128], f32, tag=f"pyy{t}", bufs=1,
                                  name=f"pyy{l}{t}")
                    at = nc.tensor.matmul(
                        pvx[:, 0:HP], bfA[0:5, sa + 6:sa + 12], Ty_prev[h],
                        start=True, stop=False,
                    )
                    nc.tensor.matmul(
                        pyy[:, 0:HP], bfA[0:5, sa:sa + 6], Ty_prev[h],
                        start=True, stop=False,
                    )
                    if l == 0:
                        for wi in warm_insts:
                            add_dep_helper(at.ins, wi.ins,
                                           reason="act warm before layer 0")
                    nc.tensor.matmul(
                        pvx[:, 0:HP], bfA[0:1, s1 + 6:s1 + 12], std_prev[h],
                        start=False, stop=True,
                    )
                    nc.tensor.matmul(
                        pyy[:, 0:HP], bfA[0:1, s1:s1 + 6], std_prev[h],
                        start=False, stop=False,
                    )
                    # LN1 std chain
                    sqr = wp.tile([6, HP], bf16, tag=f"sqr{t}",
                                  name=f"sqr{l}{t}")
                    nc.scalar.activation(out=sqr, in_=pvx[:, 0:HP],
                                         func=AF.Square)
                    nc.tensor.matmul(pvx[0:1, 64:64 + HP],
                                     bfA[0:6, A_PSV:A_PSV + 1], sqr,
                                     start=True, stop=True)
                    stda = wp.tile([1, HP], bf16, tag=f"sda{t}", bufs=2,
                                   name=f"sda{l}{t}")
                    nc.scalar.activation(out=stda, in_=pvx[0:1, 64:64 + HP],
                                         func=AF.Sqrt)
                    nc.tensor.matmul(pyy[:, 0:HP],
                                     bfA[0:1, A_ZM:A_ZM + 6], stda,
                                     start=False, stop=True)
                    Ta = wp.tile([6, HP], bf16, tag=f"Ta{t}", bufs=2,
                                 name=f"Ta{l}{t}")
                    nc.vector.tensor_copy(out=Ta, in_=pyy[:, 0:HP])
                    # FFN mm1: 16 chunks -> 2 psum banks
                    psf2 = [pp.tile([128, 512], f32, tag=f"pf{t}", bufs=2,
                                    name=f"psf{l}{t}_{q}") for q in range(2)]
                    for c in range(NCHUNK):
                        nc.tensor.matmul(
                            psf2[c // 8][:, HP * (c % 8):HP * (c % 8 + 1)],
                            w1sb[l][0:6, 128 * c:128 * (c + 1)],
                            Ta[0:6, :], start=True, stop=True,
                        )
                    nc.tensor.matmul(
                        pyy[0:5, 64:64 + HP],
                        bfA[0:6, A_PRE + 5 * l:A_PRE + 5 * (l + 1)],
                        Ta[0:6, :], start=True, stop=False,
                    )
                    # relu -> bf16 in [128, 256] pieces, alternating ACT/DVE
                    fq2 = [fp.tile([128, 512], bf16, tag=f"f{t}",
                                   name=f"f{l}{t}_{q}") for q in range(2)]
                    for q in range(2):
                        for p in range(2):
                            src_ = psf2[q][:, 256 * p:256 * (p + 1)]
                            dst_ = fq2[q][:, 256 * p:256 * (p + 1)]
                            if (h + q + p) % 2 == 0:
                                nc.vector.tensor_scalar(
                                    out=dst_, in0=src_, scalar1=0.0,
                                    scalar2=None, op0=ALU.max)
                            else:
                                nc.scalar.activation(out=dst_, in_=src_,
                                                     func=AF.Relu)
                    # mm2 accumulation into psy2 region
                    for c in range(NCHUNK):
                        nc.tensor.matmul(
                            pyy[0:5, 64:64 + HP],
                            bfB[:, B_W2 + 80 * l + 5 * c:
                                B_W2 + 80 * l + 5 * (c + 1)],
                            fq2[c // 8][:, HP * (c % 8):HP * (c % 8 + 1)],
                            start=False, stop=(c == NCHUNK - 1),
                        )
                    # LN2 chain
                    sqy = wp.tile([5, HP], bf16, tag=f"sqy{t}",
                                  name=f"sqy{l}{t}")
                    nc.scalar.activation(out=sqy, in_=pyy[0:5, 64:64 + HP],
                                         func=AF.Square)
                    Tby = wp.tile([5, HP], bf16, tag=f"Tb{t}", bufs=2,
                                  name=f"Tb{l}{t}")
                    nc.vector.tensor_copy(out=Tby, in_=pyy[0:5, 64:64 + HP])
                    nc.tensor.matmul(pvx[0:1, 128:128 + HP],
                                     bfA[0:6, A_PSV + 2:A_PSV + 3], sqr,
                                     start=True, stop=False)
                    nc.tensor.matmul(pvx[0:1, 128:128 + HP],
                                     bfA[0:5, A_PSV + 1:A_PSV + 2], sqy,
                                     start=False, stop=True)
                    stdb = wp.tile([1, HP], bf16, tag=f"sdb{t}", bufs=2,
                                   name=f"sdb{l}{t}")
                    sbi = nc.scalar.activation(out=stdb,
                                               in_=pvx[0:1, 128:128 + HP],
                                               func=AF.Sqrt)
                    if l == L - 1:
                        last_sqrts.append(sbi)
                    psvb_last[h] = pvx[0:1, 128:128 + HP]
                    Ty_prev[h] = Tby[0:5, :]
                    std_prev[h] = stdb[0:1, :]

            # ---- head (also split by halves) ----
            warm2 = cp.tile([1, 1], f32)
            w2i = nc.scalar.activation(out=warm2, in_=vconstf[0:1, 0:1],
                                       func=AF.Tanh)
            for s8i in last_sqrts:
                add_dep_helper(w2i.ins, s8i.ins,
                               reason="exp table prefetch after last sqrts")
            outs_sb = wp.tile([16, 2], f32, tag="ossb")
            zS = wp.tile([1, 2 * NP], f32, tag="zS")
            for h in range(2):
                t = tags[h]
                rec8 = wp.tile([1, HP], f32, tag=f"rc8{t}")
                nc.vector.reciprocal(out=rec8, in_=std_prev[h])
                hps = pp.tile([16, 512], f32, tag=f"pyy{t}", bufs=1,
                               name=f"hps{t}")
                nc.tensor.matmul(hps[:, 0:HP], ones_b16, rec8,
                                 start=True, stop=True)
                rbc = wp.tile([16, HP], f32, tag=f"rbc{t}")
                nc.vector.tensor_copy(out=rbc, in_=hps[:, 0:HP])

                nc.tensor.matmul(hps[:, 64:64 + HP],
                                 bfB[0:5, B_HS1:B_HS1 + 16], Ty_prev[h],
                                 start=True, stop=False)
                nc.tensor.matmul(hps[:, 64:64 + HP],
                                 bfB[0:1, B_HB1:B_HB1 + 16], std_prev[h],
                                 start=False, stop=True)
                hid = wp.tile([16, HP], bf16, tag=f"hid{t}")
                nc.vector.tensor_scalar(out=hid, in0=hps[:, 64:64 + HP],
                                        scalar1=0.0, scalar2=None, op0=ALU.max)
                nc.tensor.matmul(hps[:, 128:128 + HP],
                                 bfB[0:16, B_HS2:B_HS2 + 16], hid,
                                 start=True, stop=False)
                nc.tensor.matmul(hps[:, 128:128 + HP],
                                 bfB[0:1, B_HB2:B_HB2 + 16], std_prev[h],
                                 start=False, stop=True)
                nc.tensor.matmul(hps[:, 192:192 + HP],
                                 bfB[0:16, B_HS2 + 16:B_HS2 + 32], hid,
                                 start=True, stop=False)
                nc.tensor.matmul(hps[:, 192:192 + HP],
                                 bfB[0:1, B_HB2 + 16:B_HB2 + 32], std_prev[h],
                                 start=False, stop=True)

                s_t = wp.tile([16, HP], f32, tag=f"st{t}")
                nc.vector.scalar_tensor_tensor(
                    out=s_t, in0=hps[:, 128:128 + HP], scalar=1.0, in1=rbc,
                    op0=ALU.mult, op1=ALU.mult,
                    accum_out=outs_sb[:, h:h + 1])
                th = wp.tile([16, HP], bf16, tag=f"th{t}")
                nc.scalar.activation(out=th, in_=s_t, func=AF.Tanh)
                t_t = wp.tile([16, HP], f32, tag=f"tt{t}")
                nc.vector.tensor_tensor(out=t_t, in0=hps[:, 192:192 + HP],
                                        in1=rbc, op=ALU.mult)

                hp2 = pp.tile([B, 512], f32, tag=f"pf{t}", bufs=2,
                              name=f"hp2{t}")
                nc.tensor.matmul(hp2[0:16, 0:HP],
                                 bfB[0:16, B_TRI:B_TRI + 16], th,
                                 start=True, stop=True)
                nc.tensor.matmul(hp2[0:1, 64:64 + HP],
                                 bfB[0:16, B_TRI + 16:B_TRI + 17], th,
                                 start=True, stop=True)
                wexp = wp.tile([16, HP], f32, tag=f"wx{t}")
                nc.scalar.activation(out=wexp, in_=hp2[0:16, 0:HP],
                                     func=AF.Exp)
                wt = wp.tile([16, HP], bf16, tag=f"wt{t}")
                nc.vector.tensor_tensor(out=wt, in0=wexp, in1=t_t,
                                        op=ALU.mult)
                nc.tensor.matmul(hp2[0:1, 128:128 + HP], ones16, wt,
                                 start=True, stop=True)
                nc.scalar.activation(out=zS[0:1, HP * h:HP * (h + 1)],
                                     in_=hp2[0:1, 64:64 + HP], func=AF.Exp)
                nc.vector.tensor_copy(out=zS[0:1, NP + HP * h:NP + HP * (h + 1)],
                                      in_=hp2[0:1, 128:128 + HP])
            nc.sync.dma_start(out=outs_d[:, :], in_=outs_sb)
            nc.sync.dma_start(out=outz_d[:, :], in_=zS)

    nc.finalize()
    return nc


def _fold_inputs(inp):
    """Host-side weight folding (float64 for precision, cast at the end)."""
    C = np.eye(D) - np.ones((D, D)) / D
    g = lambda k: np.asarray(inp[k], dtype=np.float64)
    wqkv, bqkv, wo, bo = g("wqkv"), g("bqkv"), g("wo"), g("bo")
    w1, b1, w2, b2 = g("w1"), g("b1"), g("w2"), g("b2")
    ln1w, ln1b, ln2w, ln2b = g("ln1w"), g("ln1b"), g("ln2w"), g("ln2b")

    bfA = np.zeros((6, A_COLS), np.float64)
    bfB = np.zeros((128, B_COLS), np.float64)
    w1l = np.zeros((L, 6, FF), np.float64)

    for l in range(L):
        Dl = np.diag(ln2w[l - 1]) if l > 0 else np.eye(D)
        el = ln2b[l - 1] if l > 0 else np.zeros(D)
        wv = wqkv[l][2 * D:3 * D, :]
        bv = bqkv[l][2 * D:3 * D]
        A0 = np.eye(D) + wo[l] @ wv
        c_attn = wo[l] @ bv + bo[l]
        M = np.zeros((D, 6))
        M[:, 0:5] = C @ A0 @ Dl
        M[:, 5] = C @ (A0 @ el + c_attn)
        # attn stationaries: P6 rows 0-4 = y1 = M w (row 5 zero col);
        # r rows = L w.  Main part over u_prev, rank-1 row over std_prev.
        G = M.T @ M / D
        G[5, 5] += EPS
        ev, Q = np.linalg.eigh(G)
        Lm = (Q * np.sqrt(np.maximum(ev, 0.0))[None, :]) @ Q.T
        sa = A_ATTN + 12 * l
        s1 = A_ATTN1 + 12 * l
        bfA[0:5, sa:sa + 5] = M[:, 0:5].T          # P6 main cols 0-4
        bfA[0:5, sa + 6:sa + 12] = Lm[:, 0:5].T    # r main cols
        bfA[0, s1:s1 + 5] = M[:, 5]                # P6 rank-1 row
        bfA[0, s1 + 6:s1 + 12] = Lm[:, 5]          # r rank-1 row
        # pre stationary: rows 0-4 = (C diag(ln1w)).T ; row 5 = C(ln1b+b2)
        pre = np.zeros((6, 5))
        pre[0:5, :] = (C @ np.diag(ln1w[l])).T
        pre[5, :] = C @ (ln1b[l] + b2[l])
        bfA[0:6, A_PRE + 5 * l:A_PRE + 5 * (l + 1)] = pre
        # w1+b1 rows
        w1l[l, 0:5, :] = (w1[l] * ln1w[l][None, :]).T
        w1l[l, 5, :] = b1[l] + w1[l] @ ln1b[l]
        # w2 chunks: [128, 5] at col 80l+5c
        w2full = (C @ w2[l]).T                      # [FF, 5]
        for c in range(NCHUNK):
            bfB[:, B_W2 + 80 * l + 5 * c:B_W2 + 80 * l + 5 * (c + 1)] = (
                w2full[128 * c:128 * (c + 1), :])

    # psv stationaries + zmask
    bfA[:, A_PSV] = 1.0
    bfA[:, A_PSV + 1] = [.2, .2, .2, .2, .2, 0.0]
    bfA[:, A_PSV + 2] = EPS
    bfA[0, A_ZM + 5] = 1.0

    # head: sf folded on host
    f0w1, f0b1 = g("f0w1"), g("f0b1")
    f0w2, f0b2 = g("f0w2"), g("f0b2")
    sf = float(np.exp(np.float32(np.asarray(inp["sfac"])[0])))
    D8 = np.diag(ln2w[L - 1])
    e8 = ln2b[L - 1]
    bfB[0:5, B_HS1:B_HS1 + 16] = (f0w1 @ D8).T
    bfB[0, B_HB1:B_HB1 + 16] = f0b1 + f0w1 @ e8
    bfB[0:16, B_HS2:B_HS2 + 16] = f0w2.T[:, 0:16] / sf   # s-half, /sf
    bfB[0:16, B_HS2 + 16:B_HS2 + 32] = f0w2.T[:, 16:32]  # t-half
    bfB[0, B_HB2:B_HB2 + 16] = f0b2[0:16] / sf
    bfB[0, B_HB2 + 16:B_HB2 + 32] = f0b2[16:32]
    tri = np.zeros((16, 17))
    for j in range(16):
        tri[j + 1:16, j] = sf                       # sum_{k>j} * sf
    tri[:, 16] = sf
    bfB[0:16, B_TRI:B_TRI + 17] = tri

    # positional tokens, exactly as the reference builds them (fp32 ops)
    xs = (np.arange(W, dtype=np.float32) / np.float32(1e4)).astype(np.float32)
    ys = (np.arange(H, dtype=np.float32) / np.float32(1e4)).astype(np.float32)
    sinx = np.broadcast_to(np.sin(xs)[None, :], (H, W)).reshape(N)
    cosx = np.broadcast_to(np.cos(xs)[None, :], (H, W)).reshape(N)
    siny = np.broadcast_to(np.sin(ys)[:, None], (H, W)).reshape(N)
    cosy = np.broadcast_to(np.cos(ys)[:, None], (H, W)).reshape(N)
    tok = np.stack(
        [-np.ones(N, np.float32), sinx, cosx, siny, cosy], axis=0
    )                                               # [5, N]
    xflat = np.asarray(inp["x"], dtype=np.float32)[:, 0].reshape(B, N)

    return bfA, bfB, w1l, tok, xflat, sf


def get_program():
    global _PROG
    if _PROG is None:
        _PROG = _build_program()
    return _PROG


def make_in_maps(inputs):
    import ml_dtypes

    bfA, bfB, w1l, tok, xflat, sf = _fold_inputs(inputs)
    w1c = {f"w1_{l}": np.ascontiguousarray(w1l[l]).astype(ml_dtypes.bfloat16)
           for l in range(L)}
    in_maps = []
    for core in range(NCORES):
        sl = slice(core * NP, (core + 1) * NP)
        a = bfA.copy()
        a[0:5, A_TINIT:A_TINIT + NP] = tok[:, sl]
        b = bfB.copy()
        b[0:B, B_XSB:B_XSB + NP] = xflat[:, sl]
        m = dict(w1c)
        m["bfA"] = np.ascontiguousarray(a).astype(ml_dtypes.bfloat16)
        m["bfB"] = np.ascontiguousarray(b).astype(ml_dtypes.bfloat16)
        in_maps.append(m)
    return in_maps, sf


def combine_outputs(outs, outzs, sf, xflat):
    """per-core s-sums [16,2] (already /sf) and (eS|T) rows -> scalar.
    z = eS * x + T is recomposed on the host per pixel column."""
    s_tot = 0.0
    q_tot = 0.0
    for core, (o, oz) in enumerate(zip(outs, outzs)):
        s_tot += np.asarray(o, dtype=np.float64).sum()
        oz = np.asarray(oz, dtype=np.float64).reshape(2 * N // NCORES)
        eS, T = oz[:N // NCORES], oz[N // NCORES:]
        xs = xflat[:, core * NP:(core + 1) * NP].astype(np.float64)
        z = eS[None, :] * xs + T[None, :]
        q_tot += (z ** 2).sum()
    sldj = B * sf * s_tot - 0.5 * q_tot - B * N * 0.5 * np.log(2.0 * np.pi)
    return np.array(-sldj, dtype=np.float32)


def kernel(**inputs):
    from concourse.bass_utils import run_bass_kernel_spmd

    nc = get_program()
    in_maps, sf = make_in_maps(inputs)
    xflat = np.asarray(inputs["x"], dtype=np.float32)[:, 0].reshape(B, N)
    res = run_bass_kernel_spmd(nc, in_maps, core_ids=list(range(NCORES)))
    return combine_outputs([r["outs"] for r in res.results],
                           [r["outz"] for r in res.results], sf, xflat)
